# revision 44
# baseline (speedup 1.0000x reference)
"""Bass/Trainium2 kernel for GQA decode attention (fused K-projection form).

Reference computation:
  x = x_pre[:, -1, :]                               # [16, 4096]
  xq = (x @ wq.T) -> [b, 32, 128]
  qt[b,h,:] = xq[b,h,:] @ wk[kv(h)*128:+128, :]     # [b, 32, 4096]
  scores = qt . x_pre / sqrt(128)                   # [b, 32, 2048]
  attn = softmax_t(scores)
  ctx[b,h,:] = sum_t attn[b,h,t] * x_pre[b,t,:]     # [b, 32, 4096]  (lazy-V)
  out[b,h,d] = sum_D ctx[b,h,D] * wv[kv(h)*128+d,D] # [b, 32, 128]
  y = out.flat @ wo.T                               # [16, 4096]

Sharding (8 cores): batch-parallel attention (2 batches/core) +
head-parallel projections (4 heads = 1 kv group/core), exchanged with
AllToAll collectives.  All device data is bf16 (f32 PSUM accumulation);
weights are pre-transposed on the host into the layouts the PE consumes,
and the big matmuls are arranged stationary-heavy (large lhsT, narrow
moving operand) so PE streaming cost is minimized.
"""

import math

import numpy as np
import ml_dtypes

import concourse.bass as bass
import concourse.mybir as mybir
import concourse.tile as tile
from concourse import bacc
from concourse.bass_utils import run_bass_kernel_spmd
from concourse.masks import make_identity
from concourse.tile import add_dep_helper

F32 = mybir.dt.float32
BF16 = mybir.dt.bfloat16
NPBF = ml_dtypes.bfloat16

NC = 8
BSZ = 16
SEQ = 2048
DIM = 4096
NH = 32
HD = 128
B_LOC = 2        # batches per core
HL = 4           # local heads per core (= one kv group)
N_KV = 8
NT = SEQ // 128  # 16 t-tiles per batch
NDC = DIM // 128 # 32 D-chunks
SCALE = 1.0 / math.sqrt(HD)


def build_program(debug=False, nocc=False, noattn=False, notrans=False, nocopy=False):
    nc = bacc.Bacc("TRN2", target_bir_lowering=False, debug=False)

    xp = nc.dram_tensor("xp", [B_LOC, SEQ, DIM], BF16, kind="ExternalInput")
    # xlT[p, c, b] = x_pre[b, -1, c*128+p]
    xlT = nc.dram_tensor("xlT", [128, NDC * BSZ], BF16, kind="ExternalInput")
    # wqT[p, c*512 + h*128 + o] = wq[512r + h*128 + o, c*128 + p]
    wqT = nc.dram_tensor("wqT", [128, NDC * HL * HD], BF16,
                         kind="ExternalInput")
    # wk_s = wk * SCALE  (full, natural [kv*128+d, D])
    wk = nc.dram_tensor("wk", [N_KV * HD, DIM], BF16, kind="ExternalInput")
    # wvT[p, c*128 + d] = wv[128r + d, c*128 + p]
    wvT = nc.dram_tensor("wvT", [128, NDC * HD], BF16, kind="ExternalInput")
    # woT[p, h*4096 + jc*128 + j] = wo[jc*128 + j, 512r + h*128 + p]
    woT = nc.dram_tensor("woT", [128, HL * DIM], BF16, kind="ExternalInput")
    # yT[p, b*256 + jc*8 + s] = y_partial[2s+b, jc*128+p]
    yT = nc.dram_tensor("yT", [128, B_LOC * NDC * NC], BF16,
                        kind="ExternalOutput")
    if debug:
        dbg_xq = nc.dram_tensor("dbg_xq", [128, 64], BF16, kind="ExternalOutput")
        dbg_qt = nc.dram_tensor("dbg_qt", [64, DIM], BF16, kind="ExternalOutput")
        dbg_qtT = nc.dram_tensor("dbg_qtT", [128, B_LOC * NDC * NH],
                                 BF16, kind="ExternalOutput")
        dbg_ctx = nc.dram_tensor("dbg_ctx", [NH, B_LOC * DIM], BF16,
                                 kind="ExternalOutput")
        dbg_out = nc.dram_tensor("dbg_out", [NH, B_LOC * HD], BF16,
                                 kind="ExternalOutput")

    rg = [list(range(NC))]
    vs_engines = None  # round-robin copy engines, set below

    with tile.TileContext(nc) as tc:
        with (
            tc.tile_pool(name="persist", bufs=1) as pers,
            tc.tile_pool(name="dram", bufs=1, space="DRAM") as dram,
            tc.tile_pool(name="xpool", bufs=6) as xpool,
            tc.tile_pool(name="xTpool", bufs=3) as xTpool,
            tc.tile_pool(name="attn", bufs=8) as apool,
            tc.tile_pool(name="small", bufs=2) as smallp,
            tc.tile_pool(name="ctxsb", bufs=1) as ctxsbp,
            tc.tile_pool(name="pC", bufs=1) as pC,
            tc.tile_pool(name="pCw", bufs=1) as pCw,
            tc.tile_pool(name="tps", bufs=3, space="PSUM") as tps,
            tc.tile_pool(name="scps", bufs=1, space="PSUM") as scps,
                                    tc.tile_pool(name="ctxps", bufs=1, space="PSUM") as ctxps,
            tc.tile_pool(name="sumps", bufs=1, space="PSUM") as sumps,
            tc.tile_pool(name="miscps", bufs=1, space="PSUM") as miscps,
        ):
            fps = miscps
            pCps = miscps
            yps = miscps
            ident = pers.tile([128, 128], BF16)
            make_identity(nc, ident)
            ones_bf = pers.tile([128, 1], BF16)
            nc.vector.memset(ones_bf, 1.0)

            a2a1_in = dram.tile([BSZ, 512], BF16)
            a2a1_out = dram.tile([BSZ, 512], BF16)
            a2a2_in = [dram.tile([NC * HL, DIM], BF16, name=f"a2a2i{b}")
                       for b in range(B_LOC)]
            a2a2_out = [dram.tile([NC * HL, DIM], BF16, name=f"a2a2o{b}")
                        for b in range(B_LOC)]

            # ---------------- Phase A: xq (head-sharded) -> tiny AllToAll
            qtT_all = pers.tile([128, B_LOC * NDC * NH], BF16, name="qtTall")
            qtT = [qtT_all[:, b * NDC * NH:(b + 1) * NDC * NH]
                   for b in range(B_LOC)]
            stage1 = []
            with (
                tc.tile_pool(name="pA", bufs=1) as pA,
                tc.tile_pool(name="pAw", bufs=2) as pAw,
            ):
                xlT_sb = pA.tile([128, NDC * BSZ], BF16)
                nc.sync.dma_start(out=xlT_sb, in_=xlT[:, :])
                wq_pieces = []
                for q in range(4):
                    wq_q = pAw.tile([128, 8 * HL * HD], BF16, tag="wqq",
                                    name=f"wqq{q}")
                    nc.sync.dma_start(
                        out=wq_q, in_=wqT[:, q * 4096:(q + 1) * 4096])
                    wq_pieces.append(wq_q)
                # xq[b, o] for the local 512-wide o-slice; one PSUM chain
                xq_psf = scps.tile([128, 512], F32, tag="sc")
                xq_ps = xq_psf[0:BSZ]
                for q in range(4):
                    for k in range(8):
                        c = q * 8 + k
                        nc.tensor.matmul(
                            xq_ps,
                            xlT_sb[:, c * BSZ:(c + 1) * BSZ],
                            wq_pieces[q][:, k * 512:(k + 1) * 512],
                            start=(c == 0), stop=(c == NDC - 1))
                xq_sb = pA.tile([BSZ, 512], BF16)
                nc.scalar.copy(out=xq_sb, in_=xq_ps)
                d = nc.scalar.dma_start(out=a2a1_in[:, :], in_=xq_sb)
                stage1.append(d)

            cc1 = None
            if not nocc:
                cc1 = nc.gpsimd.collective_compute(
                    "AllToAll", mybir.AluOpType.bypass,
                    ins=[a2a1_in.opt()], outs=[a2a1_out.opt()],
                    replica_groups=rg)
                for d in stage1:
                    add_dep_helper(cc1.ins, d.ins, reason="a2a1 input ready")

            # qtT[b][p=D, c*32+h] = sum_d xq[2r+b, h*128+d] * wk_s[h*128+d, c*128+p]
            with (
                tc.tile_pool(name="qn", bufs=1) as qn,
                tc.tile_pool(name="wkp", bufs=8) as wkp,
            ):
                # wk pieces per kv group, streamed (SP queue, after wqT)
                wk_dmas = []
                wk_pieces = []
                for kv in range(N_KV):
                    wkq = wkp.tile([HD, DIM], BF16, tag="wkp",
                                   name=f"wk{kv}")
                    dk = nc.sync.dma_start(
                        out=wkq, in_=wk[kv * HD:(kv + 1) * HD, :])

                    wk_dmas.append(dk)
                    wk_pieces.append(wkq)
                xq_loc = qn.tile([B_LOC, DIM], BF16)
                xql_dmas = []
                av = a2a1_out.rearrange("(sq bl) o -> bl sq o", sq=NC)
                for bl in range(B_LOC):
                    d = nc.scalar.dma_start(
                        out=xq_loc[bl:bl + 1].rearrange(
                            "p (sq o) -> p sq o", sq=NC),
                        in_=av[bl])
                    xql_dmas.append(d)
                    if cc1 is not None:
                        add_dep_helper(d.ins, cc1.ins, reason="a2a1 done")
                # xqT2[p=d, 2*hg+bl] via PE transposes
                xqT2_psf = miscps.tile([128, 512], BF16, tag="ctp")
                for c in range(NDC):
                    nc.tensor.transpose(
                        xqT2_psf[:, c * 2:(c + 1) * 2],
                        xq_loc[:, c * 128:(c + 1) * 128],
                        ident[0:B_LOC, 0:B_LOC])
                xqT2_sb = qn.tile([128, NDC * B_LOC], BF16)
                nc.scalar.copy(out=xqT2_sb, in_=xqT2_psf[:, 0:NDC * B_LOC])
                # per kv: qtT chunks [128 D, (c, h, bl)]
                for kv in range(N_KV):
                    if kv % 2 == 0:
                        qt_ps = scps.tile([128, 512], F32, tag="sc")
                    else:
                        qt_ps = miscps.tile([128, 512], F32, tag="ctp")
                    qp = qt_ps.rearrange("p (c h bl) -> p c h bl", c=NDC, h=HL)
                    for c in range(NDC):
                        nc.tensor.matmul(
                            qt_ps[:, c * 8:(c + 1) * 8],
                            wk_pieces[kv][:, c * 128:(c + 1) * 128],
                            xqT2_sb[:, 8 * kv:8 * (kv + 1)],
                            start=True, stop=True)
                    qall = qtT_all.rearrange("p (bl c hh) -> p bl c hh",
                                             bl=B_LOC, c=NDC)
                    nc.scalar.copy(
                        out=qall[:, :, :, 4 * kv:4 * (kv + 1)],
                        in_=qt_ps[:, 0:256].rearrange(
                            "p (c h bl) -> p bl c h", c=NDC, h=HL))
                if debug:
                    for b in range(B_LOC):
                        nc.sync.dma_start(
                            out=dbg_qtT[:, b * NDC * NH:(b + 1) * NDC * NH],
                            in_=qtT[b])

            # ---------------- Phase B: streaming attention per local batch
            cc2 = [None, None]
            if True:
                xdmas = []
                import os
                _CP = os.environ.get("XTCOPY", "3d1a")
                def xt_copy(g, out, in_):
                    # GPSIMD cannot read PSUM; split PSUM->SBUF copies
                    # between DVE and ACT.
                    if _CP == "alldve":
                        nc.vector.tensor_copy(out=out, in_=in_)
                    elif _CP == "2d2a":
                        if g in (1, 3):
                            nc.scalar.copy(out=out, in_=in_)
                        else:
                            nc.vector.tensor_copy(out=out, in_=in_)
                    elif _CP == "split":
                        if g in (1, 3):
                            nc.scalar.copy(out=out[:, 0:512], in_=in_[:, 0:512])
                            nc.vector.tensor_copy(out=out[:, 512:1024],
                                                  in_=in_[:, 512:1024])
                        else:
                            nc.vector.tensor_copy(out=out, in_=in_)
                    else:
                        if g == 1:
                            nc.scalar.copy(out=out, in_=in_)
                        else:
                            nc.vector.tensor_copy(out=out, in_=in_)
                fin_state = {}
                fin2_state = {}

                def finalize_part1(bb, ctx_ps_b, sumT_ps_b):
                    sumT_sb = smallp.tile([NH, 1], F32, tag="ssum")
                    nc.vector.tensor_copy(out=sumT_sb, in_=sumT_ps_b)
                    recip = smallp.tile([NH, 1], F32, tag="recip")
                    nc.vector.reciprocal(out=recip, in_=sumT_sb)
                    ctxT_sb = ctxsbp.tile([128, NDC * NH], BF16, tag="ctxT")
                    for half in range(2):
                        nc.scalar.copy(
                            out=ctxT_sb[:, half * 512:(half + 1) * 512],
                            in_=ctx_ps_b[:, half * 512:(half + 1) * 512])
                    fin_state[bb] = (ctxT_sb, recip)

                def finalize_part2_groups(bb, groups, state):
                    last = (bb == B_LOC - 1)
                    ctxT_sb, recip = fin_state[bb]
                    if "ctx_sb" not in state:
                        state["ctx_sb"] = ctxsbp.tile([NH, DIM], BF16,
                                                      tag="ctxn",
                                                      name=f"ctxn{bb}")
                    ctx_sb = state["ctx_sb"]
                    for g in groups:
                        tp2f = tps.tile([128, 1024], BF16, tag="xtp")
                        tp2 = tp2f[0:NH]
                        for k in range(8):
                            c = g * 8 + k
                            nc.tensor.transpose(
                                tp2[:, k * 128:(k + 1) * 128],
                                ctxT_sb[:, c * NH:(c + 1) * NH],
                                ident)
                        if last and g % 2 == 1:
                            nc.scalar.mul(
                                out=ctx_sb[:, g * 1024:(g + 1) * 1024],
                                in_=tp2, mul=recip)
                        else:
                            nc.vector.tensor_scalar_mul(
                                ctx_sb[:, g * 1024:(g + 1) * 1024], tp2, recip)

                def finalize_part2_finish(bb, state):
                    last = (bb == B_LOC - 1)
                    fin_state.pop(bb)
                    ctx_sb = state["ctx_sb"]
                    if debug:
                        nc.sync.dma_start(
                            out=dbg_ctx[:, bb * DIM:(bb + 1) * DIM],
                            in_=ctx_sb)
                    if last:
                        d = nc.scalar.dma_start(out=a2a2_in[bb][:, :],
                                                in_=ctx_sb)
                    else:
                        d = nc.gpsimd.dma_start(out=a2a2_in[bb][:, :],
                                                in_=ctx_sb)
                    if not nocc:
                        cc2[bb] = nc.gpsimd.collective_compute(
                            "AllToAll", mybir.AluOpType.bypass,
                            ins=[a2a2_in[bb].opt()], outs=[a2a2_out[bb].opt()],
                            replica_groups=rg)
                        add_dep_helper(cc2[bb].ins, d.ins,
                                       reason="a2a2 input ready")

                for b in range(B_LOC):
                    ctx_ps = ctxps.tile([128, NDC * NH], F32, tag="ctx")
                    sumT_ps = sumps.tile([NH, 1], F32, tag="sumT")

                    def emit_attn(tt, x_sb, xT_sb):
                        sc_full = scps.tile([128, 512], F32, tag="sc")
                        sc_ps = sc_full[:, 0:NH]
                        for c in range(NDC):
                            nc.tensor.matmul(
                                sc_ps,
                                xT_sb[:, c * 128:(c + 1) * 128],
                                qtT[b][:, c * NH:(c + 1) * NH],
                                start=(c == 0), stop=(c == NDC - 1))
                        at_sb = apool.tile([128, NH], BF16, tag="at")
                        nc.scalar.activation(
                            out=at_sb, in_=sc_ps,
                            func=mybir.ActivationFunctionType.Exp)
                        nc.tensor.matmul(sumT_ps, at_sb, ones_bf,
                                         start=(tt == 0), stop=(tt == NT - 1))
                        bank_start = [None, None]
                        for c in range(NDC):
                            mm = nc.tensor.matmul(
                                ctx_ps[:, c * NH:(c + 1) * NH],
                                x_sb[:, c * 128:(c + 1) * 128],
                                at_sb,
                                start=(tt == 0 and c % 16 == 0),
                                stop=(tt == NT - 1),
                                skip_group_check=True)
                            if tt == 0:
                                if c % 16 == 0:
                                    bank_start[c // 16] = mm
                                else:
                                    add_dep_helper(
                                        mm.ins, bank_start[c // 16].ins,
                                        reason="bank wipe first")

                    import os as _os
                    _LAG = int(_os.environ.get("ALAG", "2"))
                    pending = []
                    for tt in range(NT):
                        x_sb = xpool.tile([128, DIM], BF16, tag="x",
                                          name=f"x{b}_{tt}")
                        xd = nc.sync.dma_start(
                            out=x_sb, in_=xp[b, tt * 128:(tt + 1) * 128, :])
                        import os as _os2
                        _XH = _os2.environ.get("XHOLD", "wk")
                        if len(xdmas) == 0 and _XH == "xql" and xql_dmas:
                            add_dep_helper(xd.ins, xql_dmas[-1].ins,
                                           reason="weights+xql first")
                        elif len(xdmas) == 0 and _XH == "wk" and wk_dmas:
                            add_dep_helper(xd.ins, wk_dmas[-1].ins,
                                           reason="wk first")
                        xdmas.append(xd)
                        xT_sb = xTpool.tile([128, DIM], BF16, tag="xT")
                        for g in range(0 if notrans else 4):
                            tp = tps.tile([128, 1024], BF16, tag="xtp")
                            for k in range(8):
                                c = g * 8 + k
                                nc.tensor.transpose(
                                    tp[:, k * 128:(k + 1) * 128],
                                    x_sb[:, c * 128:(c + 1) * 128], ident)
                            xt_copy(g, xT_sb[:, g * 1024:(g + 1) * 1024], tp)
                        if noattn:
                            continue
                        pending.append((tt, x_sb, xT_sb))
                        if len(pending) > _LAG:
                            emit_attn(*pending.pop(0))
                        if b == 1 and 0 in fin_state and tt == int(__import__('os').environ.get('FIN2', '2')):
                            finalize_part2_groups(0, [0, 1, 2, 3],
                                                  fin2_state)
                            finalize_part2_finish(0, fin2_state)
                    for p in pending:
                        emit_attn(*p)
                    finalize_part1(b, ctx_ps, sumT_ps)
                    if b == B_LOC - 1:
                        st = {}
                        finalize_part2_groups(b, [0, 1, 2, 3], st)
                        finalize_part2_finish(b, st)

                # ---------------- Phase C: output projection per batch slot
                wvT_sb = pCw.tile([128, NDC * HD], BF16)
                dwv = nc.sync.dma_start(out=wvT_sb, in_=wvT[:, :])
                woT_sb = pCw.tile([128, HL * DIM], BF16)
                dwo = nc.sync.dma_start(out=woT_sb, in_=woT[:, :])
                add_dep_helper(dwv.ins, xdmas[NT + 4].ins, reason="late wv")
                add_dep_helper(dwo.ins, xdmas[NT + 8].ins, reason="late wo")
                yT_sb = pCw.tile([128, NDC * B_LOC * NC], BF16)
                for b in range(B_LOC):
                    ctxgf = xpool.tile([128, DIM], BF16, tag="x",
                                       name=f"ctxg{b}")
                    ctxg = ctxgf[0:NH]
                    for hf in range(2):
                        d = nc.scalar.dma_start(
                            out=ctxg[:, hf * 2048:(hf + 1) * 2048],
                            in_=a2a2_out[b][:, hf * 2048:(hf + 1) * 2048])
                        if cc2[b] is not None:
                            add_dep_helper(d.ins, cc2[b].ins,
                                           reason="a2a2 done")
                    ctxgT = pC.tile([128, NDC * NH], BF16, tag="ctxgT")
                    for hf in range(2):
                        tpg = tps.tile([128, 1024], BF16, tag="xtp",
                                       name=f"tpg{b}_{hf}")
                        for k in range(16):
                            c = hf * 16 + k
                            nc.tensor.transpose(
                                tpg[:, k * 32:(k + 1) * 32],
                                ctxg[:, c * 128:(c + 1) * 128],
                                ident[0:NH, 0:NH])
                        nc.vector.tensor_copy(
                            out=ctxgT[:, hf * 512:(hf + 1) * 512],
                            in_=tpg[:, 0:512])
                    # outT[d, (s,h)] = sum_D wvT[D, d]^T ctxgT[D, (s,h)]
                    op_ps = pCps.tile([HD, NH], F32, tag="ctp")
                    for c in range(NDC):
                        nc.tensor.matmul(op_ps,
                                         wvT_sb[:, c * 128:(c + 1) * 128],
                                         ctxgT[:, c * NH:(c + 1) * NH],
                                         start=(c == 0), stop=(c == NDC - 1))
                    outT = pC.tile([128, NH], BF16, tag="outT")
                    nc.vector.tensor_copy(out=outT[0:HD], in_=op_ps)
                    # yT[j, s] = sum_h sum_d woT[d, (h, jc, j)] * outT[d, (s, h)]
                    ov = outT.rearrange("p (s h) -> p h s", h=HL)
                    y_ps = yps.tile([128, NDC * NC], F32, tag="ctp")
                    for jc in range(NDC):
                        for h in range(HL):
                            nc.tensor.matmul(
                                y_ps[:, jc * NC:(jc + 1) * NC],
                                woT_sb[:, h * DIM + jc * 128:
                                       h * DIM + (jc + 1) * 128],
                                ov[:, h, :],
                                start=(h == 0), stop=(h == HL - 1))
                    yv = yT_sb.rearrange("p (b jcs) -> b p jcs", b=B_LOC)
                    nc.vector.tensor_copy(out=yv[b], in_=y_ps)
                    nc.sync.dma_start(
                        out=yT.rearrange("p (b jcs) -> b p jcs", b=B_LOC)[b],
                        in_=yv[b])

    nc.finalize()
    return nc


_PROGRAM_CACHE = {}


def _prep_inputs(x_pre, wq, wk, wv, wo):
    """Shard + cast + pre-transpose on host. Returns in_maps for 8 cores."""
    xlT_full = np.ascontiguousarray(
        x_pre[:, -1, :].T.astype(NPBF))                    # [4096, 16]
    xlT_full = xlT_full.reshape(NDC, 128, BSZ).transpose(1, 0, 2)  # [128,c,b]
    xlT_flat = np.ascontiguousarray(xlT_full.reshape(128, NDC * BSZ))

    wk_s = (wk * SCALE).astype(NPBF)
    in_maps = []
    for r in range(NC):
        # wqT[p, c, h, o] = wq[512r + h*128 + o, c*128 + p]
        wq_sl = wq[512 * r:512 * (r + 1), :].astype(NPBF)   # [512, 4096] (h,o)xD
        wqT_r = wq_sl.reshape(HL, 128, NDC, 128).transpose(3, 2, 0, 1)
        wqT_r = np.ascontiguousarray(wqT_r.reshape(128, NDC * HL * HD))
        # wvT[p, c, d] = wv[128r + d, c*128 + p]
        wv_sl = wv[128 * r:128 * (r + 1), :].astype(NPBF)   # [128 d, 4096 D]
        wvT_r = wv_sl.reshape(128, NDC, 128).transpose(2, 1, 0)
        wvT_r = np.ascontiguousarray(wvT_r.reshape(128, NDC * HD))
        # woT[p, h, jc, j] = wo[jc*128 + j, 512r + h*128 + p]
        wo_sl = wo[:, 512 * r:512 * (r + 1)].astype(NPBF)   # [4096 j, 512 o]
        woT_r = wo_sl.reshape(NDC, 128, HL, 128).transpose(3, 2, 0, 1)
        woT_r = np.ascontiguousarray(woT_r.reshape(128, HL * DIM))
        in_maps.append({
            "xp": np.ascontiguousarray(x_pre[2 * r:2 * r + 2].astype(NPBF)),
            "xlT": xlT_flat,
            "wqT": wqT_r,
            "wk": np.ascontiguousarray(wk_s),
            "wvT": wvT_r,
            "woT": woT_r,
        })
    return in_maps


def kernel(x_pre, wq, wk, wv, wo, _trace=False, _tmpdir=None, _debug=False):
    x_pre = np.asarray(x_pre, dtype=np.float32)
    wq = np.asarray(wq, dtype=np.float32)
    wk = np.asarray(wk, dtype=np.float32)
    wv = np.asarray(wv, dtype=np.float32)
    wo = np.asarray(wo, dtype=np.float32)

    key = "nc_dbg" if _debug else "nc"
    if key not in _PROGRAM_CACHE:
        _PROGRAM_CACHE[key] = build_program(debug=_debug)
        _PROGRAM_CACHE["nc"] = _PROGRAM_CACHE[key]
    nc = _PROGRAM_CACHE[key]

    in_maps = _prep_inputs(x_pre, wq, wk, wv, wo)

    kwargs = {}
    if _trace:
        kwargs = dict(trace=True, trace_cores=[0])
    if _tmpdir is not None:
        kwargs["tmpdir"] = _tmpdir
    res = run_bass_kernel_spmd(nc, in_maps, core_ids=list(range(NC)), **kwargs)

    y = np.zeros((BSZ, DIM), np.float64)
    for r in range(NC):
        yT_r = np.asarray(res.results[r]["yT"], np.float32)
        yT_r = yT_r.reshape(128, B_LOC, NDC, NC)
        # y[2s+b, jc*128+p] += yT_r[p, b, jc, s]
        y += yT_r.transpose(3, 1, 2, 0).reshape(BSZ, DIM)
    if _debug:
        _PROGRAM_CACHE["dbg"] = res
    if _trace:
        print("HW exec time:", res.exec_time_ns, "ns")
    return y.astype(np.float32).reshape(BSZ, 1, DIM)



# revision 58
# speedup vs baseline: 1.0158x; 1.0158x over previous
"""Bass/Trainium2 kernel for GQA decode attention (fused K-projection form).

Reference computation:
  x = x_pre[:, -1, :]                               # [16, 4096]
  xq = (x @ wq.T) -> [b, 32, 128]
  qt[b,h,:] = xq[b,h,:] @ wk[kv(h)*128:+128, :]     # [b, 32, 4096]
  scores = qt . x_pre / sqrt(128)                   # [b, 32, 2048]
  attn = softmax_t(scores)
  ctx[b,h,:] = sum_t attn[b,h,t] * x_pre[b,t,:]     # [b, 32, 4096]  (lazy-V)
  out[b,h,d] = sum_D ctx[b,h,D] * wv[kv(h)*128+d,D] # [b, 32, 128]
  y = out.flat @ wo.T                               # [16, 4096]

Sharding (8 cores): batch-parallel attention (2 batches/core) +
head-parallel projections (4 heads = 1 kv group/core), exchanged with
AllToAll collectives.  All device data is bf16 (f32 PSUM accumulation);
weights are pre-transposed on the host into the layouts the PE consumes,
and the big matmuls are arranged stationary-heavy (large lhsT, narrow
moving operand) so PE streaming cost is minimized.
"""

import math

import numpy as np
import ml_dtypes

import concourse.bass as bass
import concourse.mybir as mybir
import concourse.tile as tile
from concourse import bacc
from concourse.bass_utils import run_bass_kernel_spmd
from concourse.masks import make_identity
from concourse.tile import add_dep_helper

F32 = mybir.dt.float32
BF16 = mybir.dt.bfloat16
F8 = mybir.dt.float8e4
NPBF = ml_dtypes.bfloat16
NPF8 = ml_dtypes.float8_e4m3
WK_PRESCALE = 1024.0
WQ_PRESCALE = 64.0

NC = 8
BSZ = 16
SEQ = 2048
DIM = 4096
NH = 32
HD = 128
B_LOC = 2        # batches per core
HL = 4           # local heads per core (= one kv group)
N_KV = 8
NT = SEQ // 128  # 16 t-tiles per batch
NDC = DIM // 128 # 32 D-chunks
SCALE = 1.0 / math.sqrt(HD)


def build_program(debug=False, nocc=False, noattn=False, notrans=False, nocopy=False):
    nc = bacc.Bacc("TRN2", target_bir_lowering=False, debug=False)

    xp = nc.dram_tensor("xp", [B_LOC, SEQ, DIM], BF16, kind="ExternalInput")
    # xlT[p, c, b] = x_pre[b, -1, c*128+p]
    xlT = nc.dram_tensor("xlT", [128, NDC * BSZ], BF16, kind="ExternalInput")
    # wqT[p, c*512 + h*128 + o] = wq[512r + h*128 + o, c*128 + p]
    wqT = nc.dram_tensor("wqT", [128, NDC * HL * HD], BF16,
                         kind="ExternalInput")
    # wk_s = wk * SCALE  (full, natural [kv*128+d, D])
    wk = nc.dram_tensor("wk", [N_KV * HD, DIM], BF16, kind="ExternalInput")
    # wvT[p, c*128 + d] = wv[128r + d, c*128 + p]
    wvT = nc.dram_tensor("wvT", [128, NDC * HD], BF16, kind="ExternalInput")
    # woT[p, h*4096 + jc*128 + j] = wo[jc*128 + j, 512r + h*128 + p]
    woT = nc.dram_tensor("woT", [128, HL * DIM], BF16, kind="ExternalInput")
    # yT[p, b*256 + jc*8 + s] = y_partial[2s+b, jc*128+p]
    yT = nc.dram_tensor("yT", [128, B_LOC * NDC * NC], BF16,
                        kind="ExternalOutput")
    if debug:
        dbg_xq = nc.dram_tensor("dbg_xq", [128, 64], BF16, kind="ExternalOutput")
        dbg_qt = nc.dram_tensor("dbg_qt", [64, DIM], BF16, kind="ExternalOutput")
        dbg_qtT = nc.dram_tensor("dbg_qtT", [128, B_LOC * NDC * NH],
                                 BF16, kind="ExternalOutput")
        dbg_ctx = nc.dram_tensor("dbg_ctx", [NH, B_LOC * DIM], BF16,
                                 kind="ExternalOutput")
        dbg_out = nc.dram_tensor("dbg_out", [NH, B_LOC * HD], BF16,
                                 kind="ExternalOutput")

    rg = [list(range(NC))]
    vs_engines = None  # round-robin copy engines, set below

    with tile.TileContext(nc) as tc:
        with (
            tc.tile_pool(name="persist", bufs=1) as pers,
            tc.tile_pool(name="dram", bufs=1, space="DRAM") as dram,
            tc.tile_pool(name="xpool", bufs=6) as xpool,
            tc.tile_pool(name="xTpool", bufs=3) as xTpool,
            tc.tile_pool(name="attn", bufs=8) as apool,
            tc.tile_pool(name="small", bufs=2) as smallp,
            tc.tile_pool(name="ctxsb", bufs=1) as ctxsbp,
            tc.tile_pool(name="pC", bufs=1) as pC,
            tc.tile_pool(name="pCw", bufs=1) as pCw,
            tc.tile_pool(name="tps", bufs=3, space="PSUM") as tps,
            tc.tile_pool(name="scps", bufs=1, space="PSUM") as scps,
                                    tc.tile_pool(name="ctxps", bufs=1, space="PSUM") as ctxps,
            tc.tile_pool(name="sumps", bufs=1, space="PSUM") as sumps,
            tc.tile_pool(name="miscps", bufs=1, space="PSUM") as miscps,
        ):
            fps = miscps
            pCps = miscps
            yps = miscps
            ident = pers.tile([128, 128], BF16)
            make_identity(nc, ident)
            ones_bf = pers.tile([128, 1], BF16)
            nc.vector.memset(ones_bf, 1.0)

            a2a1_in = dram.tile([BSZ, 512], BF16)
            a2a1_out = dram.tile([BSZ, 512], BF16)
            a2a2_in = [dram.tile([NC * HL, DIM], BF16, name=f"a2a2i{b}")
                       for b in range(B_LOC)]
            a2a2_out = [dram.tile([NC * HL, DIM], BF16, name=f"a2a2o{b}")
                        for b in range(B_LOC)]

            # ---------------- Phase A: xq (head-sharded) -> tiny AllToAll
            qtT_all = pers.tile([128, B_LOC * NDC * NH], BF16, name="qtTall")
            qtT = [qtT_all[:, b * NDC * NH:(b + 1) * NDC * NH]
                   for b in range(B_LOC)]
            stage1 = []
            with (
                tc.tile_pool(name="pA", bufs=1) as pA,
                tc.tile_pool(name="pAw", bufs=16) as pAw,
            ):
                xlT_sb = pA.tile([128, NDC * BSZ], BF16)
                nc.sync.dma_start(out=xlT_sb, in_=xlT[:, :])
                wq_pieces = []
                for q in range(16):
                    wq_q = pAw.tile([128, 2 * HL * HD], BF16, tag="wqq",
                                    name=f"wqq{q}")
                    nc.sync.dma_start(
                        out=wq_q, in_=wqT[:, q * 1024:(q + 1) * 1024])
                    wq_pieces.append(wq_q)
                # xq[b, o] for the local 512-wide o-slice; one PSUM chain
                xq_psf = scps.tile([128, 512], F32, tag="sc")
                xq_ps = xq_psf[0:BSZ]
                for q in range(16):
                    for k in range(2):
                        c = q * 2 + k
                        nc.tensor.matmul(
                            xq_ps,
                            xlT_sb[:, c * BSZ:(c + 1) * BSZ],
                            wq_pieces[q][:, k * 512:(k + 1) * 512],
                            start=(c == 0), stop=(c == NDC - 1))
                xq_sb = pA.tile([BSZ, 512], BF16)
                nc.scalar.copy(out=xq_sb, in_=xq_ps)
                d = nc.scalar.dma_start(out=a2a1_in[:, :], in_=xq_sb)
                stage1.append(d)

            cc1 = None
            if not nocc:
                cc1 = nc.gpsimd.collective_compute(
                    "AllToAll", mybir.AluOpType.bypass,
                    ins=[a2a1_in.opt()], outs=[a2a1_out.opt()],
                    replica_groups=rg)
                for d in stage1:
                    add_dep_helper(cc1.ins, d.ins, reason="a2a1 input ready")

            # qtT[b][p=D, c*32+h] = sum_d xq[2r+b, h*128+d] * wk_s[h*128+d, c*128+p]
            with (
                tc.tile_pool(name="qn", bufs=1) as qn,
                tc.tile_pool(name="wkp", bufs=8) as wkp,
            ):
                # wk pieces per kv group, streamed (SP queue, after wqT)
                wk_dmas = []
                wk_pieces = []
                for kv in range(N_KV):
                    wkq = wkp.tile([HD, DIM], BF16, tag="wkp",
                                   name=f"wk{kv}")
                    dk = nc.sync.dma_start(
                        out=wkq, in_=wk[kv * HD:(kv + 1) * HD, :])

                    wk_dmas.append(dk)
                    wk_pieces.append(wkq)
                xq_loc = qn.tile([B_LOC, DIM], BF16)
                xql_dmas = []
                av = a2a1_out.rearrange("(sq bl) o -> bl sq o", sq=NC)
                for bl in range(B_LOC):
                    d = nc.scalar.dma_start(
                        out=xq_loc[bl:bl + 1].rearrange(
                            "p (sq o) -> p sq o", sq=NC),
                        in_=av[bl])
                    xql_dmas.append(d)
                    if cc1 is not None:
                        add_dep_helper(d.ins, cc1.ins, reason="a2a1 done")
                # xqT2[p=d, 2*hg+bl] via PE transposes
                xqT2_psf = miscps.tile([128, 512], BF16, tag="ctp")
                for c in range(NDC):
                    nc.tensor.transpose(
                        xqT2_psf[:, c * 2:(c + 1) * 2],
                        xq_loc[:, c * 128:(c + 1) * 128],
                        ident[0:B_LOC, 0:B_LOC])
                xqT2_sb = qn.tile([128, NDC * B_LOC], BF16)
                nc.scalar.copy(out=xqT2_sb, in_=xqT2_psf[:, 0:NDC * B_LOC])
                # per kv: qtT chunks [128 D, (c, h, bl)]
                for kv in range(N_KV):
                    if kv % 2 == 0:
                        qt_ps = scps.tile([128, 512], F32, tag="sc")
                    else:
                        qt_ps = miscps.tile([128, 512], F32, tag="ctp")
                    qp = qt_ps.rearrange("p (c h bl) -> p c h bl", c=NDC, h=HL)
                    for c in range(NDC):
                        nc.tensor.matmul(
                            qt_ps[:, c * 8:(c + 1) * 8],
                            wk_pieces[kv][:, c * 128:(c + 1) * 128],
                            xqT2_sb[:, 8 * kv:8 * (kv + 1)],
                            start=True, stop=True)
                    qall = qtT_all.rearrange("p (bl c hh) -> p bl c hh",
                                             bl=B_LOC, c=NDC)
                    if kv % 2 == 0:
                        nc.vector.tensor_copy(
                            out=qall[:, :, :, 4 * kv:4 * (kv + 1)],
                            in_=qt_ps[:, 0:256].rearrange(
                                "p (c h bl) -> p bl c h", c=NDC, h=HL))
                    else:
                        nc.scalar.copy(
                            out=qall[:, :, :, 4 * kv:4 * (kv + 1)],
                            in_=qt_ps[:, 0:256].rearrange(
                                "p (c h bl) -> p bl c h", c=NDC, h=HL))
                if debug:
                    for b in range(B_LOC):
                        nc.sync.dma_start(
                            out=dbg_qtT[:, b * NDC * NH:(b + 1) * NDC * NH],
                            in_=qtT[b])

            # ---------------- Phase B: streaming attention per local batch
            cc2 = [None, None]
            if True:
                xdmas = []
                import os
                _CP = os.environ.get("XTCOPY", "3d1a")
                def xt_copy(g, out, in_):
                    # GPSIMD cannot read PSUM; split PSUM->SBUF copies
                    # between DVE and ACT.
                    if _CP == "alldve":
                        nc.vector.tensor_copy(out=out, in_=in_)
                    elif _CP == "2d2a":
                        if g in (1, 3):
                            nc.scalar.copy(out=out, in_=in_)
                        else:
                            nc.vector.tensor_copy(out=out, in_=in_)
                    elif _CP == "split":
                        if g in (1, 3):
                            nc.scalar.copy(out=out[:, 0:512], in_=in_[:, 0:512])
                            nc.vector.tensor_copy(out=out[:, 512:1024],
                                                  in_=in_[:, 512:1024])
                        else:
                            nc.vector.tensor_copy(out=out, in_=in_)
                    else:
                        if g == 1:
                            nc.scalar.copy(out=out, in_=in_)
                        else:
                            nc.vector.tensor_copy(out=out, in_=in_)
                fin_state = {}
                fin2_state = {}

                def finalize_part1(bb, ctx_ps_b, sumT_ps_b):
                    sumT_sb = smallp.tile([NH, 1], F32, tag="ssum")
                    nc.vector.tensor_copy(out=sumT_sb, in_=sumT_ps_b)
                    recip = smallp.tile([NH, 1], F32, tag="recip")
                    nc.vector.reciprocal(out=recip, in_=sumT_sb)
                    ctxT_sb = ctxsbp.tile([128, NDC * NH], BF16, tag="ctxT")
                    for half in range(2):
                        nc.scalar.copy(
                            out=ctxT_sb[:, half * 512:(half + 1) * 512],
                            in_=ctx_ps_b[:, half * 512:(half + 1) * 512])
                    fin_state[bb] = (ctxT_sb, recip)

                def finalize_part2_groups(bb, groups, state):
                    last = (bb == B_LOC - 1)
                    ctxT_sb, recip = fin_state[bb]
                    if "ctx_sb" not in state:
                        state["ctx_sb"] = ctxsbp.tile([NH, DIM], BF16,
                                                      tag="ctxn",
                                                      name=f"ctxn{bb}")
                    ctx_sb = state["ctx_sb"]
                    for g in groups:
                        tp2f = tps.tile([128, 1024], BF16, tag="xtp")
                        tp2 = tp2f[0:NH]
                        for k in range(8):
                            c = g * 8 + k
                            nc.tensor.transpose(
                                tp2[:, k * 128:(k + 1) * 128],
                                ctxT_sb[:, c * NH:(c + 1) * NH],
                                ident)
                        if last and g % 2 == 1:
                            nc.scalar.mul(
                                out=ctx_sb[:, g * 1024:(g + 1) * 1024],
                                in_=tp2, mul=recip)
                        else:
                            nc.vector.tensor_scalar_mul(
                                ctx_sb[:, g * 1024:(g + 1) * 1024], tp2, recip)

                def finalize_part2_finish(bb, state):
                    last = (bb == B_LOC - 1)
                    fin_state.pop(bb)
                    ctx_sb = state["ctx_sb"]
                    if debug:
                        nc.sync.dma_start(
                            out=dbg_ctx[:, bb * DIM:(bb + 1) * DIM],
                            in_=ctx_sb)
                    if last:
                        d = nc.scalar.dma_start(out=a2a2_in[bb][:, :],
                                                in_=ctx_sb)
                    else:
                        d = nc.gpsimd.dma_start(out=a2a2_in[bb][:, :],
                                                in_=ctx_sb)
                    if not nocc:
                        cc2[bb] = nc.gpsimd.collective_compute(
                            "AllToAll", mybir.AluOpType.bypass,
                            ins=[a2a2_in[bb].opt()], outs=[a2a2_out[bb].opt()],
                            replica_groups=rg)
                        add_dep_helper(cc2[bb].ins, d.ins,
                                       reason="a2a2 input ready")

                for b in range(B_LOC):
                    ctx_ps = ctxps.tile([128, NDC * NH], F32, tag="ctx")
                    sumT_ps = sumps.tile([NH, 1], F32, tag="sumT")

                    def emit_attn(tt, x_sb, xT_sb):
                        sc_full = scps.tile([128, 512], F32, tag="sc")
                        sc_ps = sc_full[:, 0:NH]
                        for c in range(NDC):
                            nc.tensor.matmul(
                                sc_ps,
                                xT_sb[:, c * 128:(c + 1) * 128],
                                qtT[b][:, c * NH:(c + 1) * NH],
                                start=(c == 0), stop=(c == NDC - 1))
                        at_sb = apool.tile([128, NH], BF16, tag="at")
                        nc.scalar.activation(
                            out=at_sb, in_=sc_ps,
                            func=mybir.ActivationFunctionType.Exp)
                        nc.tensor.matmul(sumT_ps, at_sb, ones_bf,
                                         start=(tt == 0), stop=(tt == NT - 1))
                        bank_start = [None, None]
                        for c in range(NDC):
                            mm = nc.tensor.matmul(
                                ctx_ps[:, c * NH:(c + 1) * NH],
                                x_sb[:, c * 128:(c + 1) * 128],
                                at_sb,
                                start=(tt == 0 and c % 16 == 0),
                                stop=(tt == NT - 1),
                                skip_group_check=True)
                            if tt == 0:
                                if c % 16 == 0:
                                    bank_start[c // 16] = mm
                                else:
                                    add_dep_helper(
                                        mm.ins, bank_start[c // 16].ins,
                                        reason="bank wipe first")

                    import os as _os
                    _LAG = int(_os.environ.get("ALAG", "2"))
                    pending = []
                    for tt in range(NT):
                        x_sb = xpool.tile([128, DIM], BF16, tag="x",
                                          name=f"x{b}_{tt}")
                        xd = nc.sync.dma_start(
                            out=x_sb, in_=xp[b, tt * 128:(tt + 1) * 128, :])
                        import os as _os2
                        _XH = _os2.environ.get("XHOLD", "wk")
                        if len(xdmas) == 0 and _XH == "xql" and xql_dmas:
                            add_dep_helper(xd.ins, xql_dmas[-1].ins,
                                           reason="weights+xql first")
                        elif len(xdmas) == 0 and _XH == "wk" and wk_dmas:
                            add_dep_helper(xd.ins, wk_dmas[-1].ins,
                                           reason="wk first")
                        xdmas.append(xd)
                        xT_sb = xTpool.tile([128, DIM], BF16, tag="xT")
                        for g in range(0 if notrans else 4):
                            tp = tps.tile([128, 1024], BF16, tag="xtp")
                            for k in range(8):
                                c = g * 8 + k
                                nc.tensor.transpose(
                                    tp[:, k * 128:(k + 1) * 128],
                                    x_sb[:, c * 128:(c + 1) * 128], ident)
                            xt_copy(g, xT_sb[:, g * 1024:(g + 1) * 1024], tp)
                        if noattn:
                            continue
                        pending.append((tt, x_sb, xT_sb))
                        if len(pending) > _LAG:
                            emit_attn(*pending.pop(0))
                        if b == 1 and 0 in fin_state and tt == int(__import__('os').environ.get('FIN2', '2')):
                            finalize_part2_groups(0, [0, 1, 2, 3],
                                                  fin2_state)
                            finalize_part2_finish(0, fin2_state)
                    for p in pending:
                        emit_attn(*p)
                    finalize_part1(b, ctx_ps, sumT_ps)
                    if b == B_LOC - 1:
                        st = {}
                        finalize_part2_groups(b, [0, 1, 2, 3], st)
                        finalize_part2_finish(b, st)

                # ---------------- Phase C: output projection per batch slot
                wvT_sb = pCw.tile([128, NDC * HD], BF16)
                dwv = nc.sync.dma_start(out=wvT_sb, in_=wvT[:, :])
                woT_sb = pCw.tile([128, HL * DIM], BF16)
                dwo = nc.sync.dma_start(out=woT_sb, in_=woT[:, :])
                import os as _os3
                _WVK = int(_os3.environ.get("WVK", str(NT + 4)))
                add_dep_helper(dwv.ins, xdmas[_WVK].ins, reason="late wv")
                add_dep_helper(dwo.ins, xdmas[_WVK + 4].ins,
                               reason="late wo")
                yT_sb = pCw.tile([128, NDC * B_LOC * NC], BF16)
                for b in range(B_LOC):
                    ctxgf = xpool.tile([128, DIM], BF16, tag="x",
                                       name=f"ctxg{b}")
                    ctxg = ctxgf[0:NH]
                    for hf in range(2):
                        d = nc.scalar.dma_start(
                            out=ctxg[:, hf * 2048:(hf + 1) * 2048],
                            in_=a2a2_out[b][:, hf * 2048:(hf + 1) * 2048])
                        if cc2[b] is not None:
                            add_dep_helper(d.ins, cc2[b].ins,
                                           reason="a2a2 done")
                    ctxgT = pC.tile([128, NDC * NH], BF16, tag="ctxgT")
                    for hf in range(2):
                        tpg = tps.tile([128, 1024], BF16, tag="xtp",
                                       name=f"tpg{b}_{hf}")
                        for k in range(16):
                            c = hf * 16 + k
                            nc.tensor.transpose(
                                tpg[:, k * 32:(k + 1) * 32],
                                ctxg[:, c * 128:(c + 1) * 128],
                                ident[0:NH, 0:NH])
                        nc.vector.tensor_copy(
                            out=ctxgT[:, hf * 512:(hf + 1) * 512],
                            in_=tpg[:, 0:512])
                    # outT[d, (s,h)] = sum_D wvT[D, d]^T ctxgT[D, (s,h)]
                    op_ps = pCps.tile([HD, NH], F32, tag="ctp")
                    for c in range(NDC):
                        nc.tensor.matmul(op_ps,
                                         wvT_sb[:, c * 128:(c + 1) * 128],
                                         ctxgT[:, c * NH:(c + 1) * NH],
                                         start=(c == 0), stop=(c == NDC - 1))
                    outT = pC.tile([128, NH], BF16, tag="outT")
                    nc.vector.tensor_copy(out=outT[0:HD], in_=op_ps)
                    # yT[j, s] = sum_h sum_d woT[d, (h, jc, j)] * outT[d, (s, h)]
                    ov = outT.rearrange("p (s h) -> p h s", h=HL)
                    y_ps = yps.tile([128, NDC * NC], F32, tag="ctp")
                    for jc in range(NDC):
                        for h in range(HL):
                            nc.tensor.matmul(
                                y_ps[:, jc * NC:(jc + 1) * NC],
                                woT_sb[:, h * DIM + jc * 128:
                                       h * DIM + (jc + 1) * 128],
                                ov[:, h, :],
                                start=(h == 0), stop=(h == HL - 1))
                    yv = yT_sb.rearrange("p (b jcs) -> b p jcs", b=B_LOC)
                    nc.vector.tensor_copy(out=yv[b], in_=y_ps)
                    nc.sync.dma_start(
                        out=yT.rearrange("p (b jcs) -> b p jcs", b=B_LOC)[b],
                        in_=yv[b])

    nc.finalize()
    return nc


_PROGRAM_CACHE = {}


def _prep_inputs(x_pre, wq, wk, wv, wo):
    """Shard + cast + pre-transpose on host. Returns in_maps for 8 cores."""
    xlT_full = np.ascontiguousarray(
        x_pre[:, -1, :].T.astype(NPBF))                    # [4096, 16]
    xlT_full = xlT_full.reshape(NDC, 128, BSZ).transpose(1, 0, 2)  # [128,c,b]
    xlT_flat = np.ascontiguousarray(xlT_full.reshape(128, NDC * BSZ))

    wk_s = (wk * SCALE).astype(NPBF)
    in_maps = []
    for r in range(NC):
        # wqT[p, c, h, o] = wq[512r + h*128 + o, c*128 + p]
        wq_sl = wq[512 * r:512 * (r + 1), :].astype(NPBF)   # [512, 4096]
        wqT_r = wq_sl.reshape(HL, 128, NDC, 128).transpose(3, 2, 0, 1)
        wqT_r = np.ascontiguousarray(wqT_r.reshape(128, NDC * HL * HD))
        # wvT[p, c, d] = wv[128r + d, c*128 + p]
        wv_sl = wv[128 * r:128 * (r + 1), :].astype(NPBF)   # [128 d, 4096 D]
        wvT_r = wv_sl.reshape(128, NDC, 128).transpose(2, 1, 0)
        wvT_r = np.ascontiguousarray(wvT_r.reshape(128, NDC * HD))
        # woT[p, h, jc, j] = wo[jc*128 + j, 512r + h*128 + p]
        wo_sl = wo[:, 512 * r:512 * (r + 1)].astype(NPBF)   # [4096 j, 512 o]
        woT_r = wo_sl.reshape(NDC, 128, HL, 128).transpose(3, 2, 0, 1)
        woT_r = np.ascontiguousarray(woT_r.reshape(128, HL * DIM))
        in_maps.append({
            "xp": np.ascontiguousarray(x_pre[2 * r:2 * r + 2].astype(NPBF)),
            "xlT": xlT_flat,
            "wqT": wqT_r,
            "wk": np.ascontiguousarray(wk_s),
            "wvT": wvT_r,
            "woT": woT_r,
        })
    return in_maps


def kernel(x_pre, wq, wk, wv, wo, _trace=False, _tmpdir=None, _debug=False):
    x_pre = np.asarray(x_pre, dtype=np.float32)
    wq = np.asarray(wq, dtype=np.float32)
    wk = np.asarray(wk, dtype=np.float32)
    wv = np.asarray(wv, dtype=np.float32)
    wo = np.asarray(wo, dtype=np.float32)

    key = "nc_dbg" if _debug else "nc"
    if key not in _PROGRAM_CACHE:
        _PROGRAM_CACHE[key] = build_program(debug=_debug)
        _PROGRAM_CACHE["nc"] = _PROGRAM_CACHE[key]
    nc = _PROGRAM_CACHE[key]

    in_maps = _prep_inputs(x_pre, wq, wk, wv, wo)

    kwargs = {}
    if _trace:
        kwargs = dict(trace=True, trace_cores=[0])
    if _tmpdir is not None:
        kwargs["tmpdir"] = _tmpdir
    res = run_bass_kernel_spmd(nc, in_maps, core_ids=list(range(NC)), **kwargs)

    y = np.zeros((BSZ, DIM), np.float64)
    for r in range(NC):
        yT_r = np.asarray(res.results[r]["yT"], np.float32)
        yT_r = yT_r.reshape(128, B_LOC, NDC, NC)
        # y[2s+b, jc*128+p] += yT_r[p, b, jc, s]
        y += yT_r.transpose(3, 1, 2, 0).reshape(BSZ, DIM)
    if _debug:
        _PROGRAM_CACHE["dbg"] = res
    if _trace:
        print("HW exec time:", res.exec_time_ns, "ns")
    return y.astype(np.float32).reshape(BSZ, 1, DIM)



# revision 61
# speedup vs baseline: 1.0186x; 1.0027x over previous
"""Bass/Trainium2 kernel for GQA decode attention (fused K-projection form).

Reference computation:
  x = x_pre[:, -1, :]                               # [16, 4096]
  xq = (x @ wq.T) -> [b, 32, 128]
  qt[b,h,:] = xq[b,h,:] @ wk[kv(h)*128:+128, :]     # [b, 32, 4096]
  scores = qt . x_pre / sqrt(128)                   # [b, 32, 2048]
  attn = softmax_t(scores)
  ctx[b,h,:] = sum_t attn[b,h,t] * x_pre[b,t,:]     # [b, 32, 4096]  (lazy-V)
  out[b,h,d] = sum_D ctx[b,h,D] * wv[kv(h)*128+d,D] # [b, 32, 128]
  y = out.flat @ wo.T                               # [16, 4096]

Sharding (8 cores): batch-parallel attention (2 batches/core) +
head-parallel projections (4 heads = 1 kv group/core), exchanged with
AllToAll collectives.  All device data is bf16 (f32 PSUM accumulation);
weights are pre-transposed on the host into the layouts the PE consumes,
and the big matmuls are arranged stationary-heavy (large lhsT, narrow
moving operand) so PE streaming cost is minimized.
"""

import math

import numpy as np
import ml_dtypes

import concourse.bass as bass
import concourse.mybir as mybir
import concourse.tile as tile
from concourse import bacc
from concourse.bass_utils import run_bass_kernel_spmd
from concourse.masks import make_identity
from concourse.tile import add_dep_helper

F32 = mybir.dt.float32
BF16 = mybir.dt.bfloat16
F8 = mybir.dt.float8e4
NPBF = ml_dtypes.bfloat16
NPF8 = ml_dtypes.float8_e4m3
WK_PRESCALE = 1024.0
WQ_PRESCALE = 64.0

NC = 8
BSZ = 16
SEQ = 2048
DIM = 4096
NH = 32
HD = 128
B_LOC = 2        # batches per core
HL = 4           # local heads per core (= one kv group)
N_KV = 8
NT = SEQ // 128  # 16 t-tiles per batch
NDC = DIM // 128 # 32 D-chunks
SCALE = 1.0 / math.sqrt(HD)


def build_program(debug=False, nocc=False, noattn=False, notrans=False, nocopy=False):
    nc = bacc.Bacc("TRN2", target_bir_lowering=False, debug=False)

    xp = nc.dram_tensor("xp", [B_LOC, SEQ, DIM], BF16, kind="ExternalInput")
    # xlT[p, c, b] = x_pre[b, -1, c*128+p]
    xlT = nc.dram_tensor("xlT", [128, NDC * BSZ], BF16, kind="ExternalInput")
    # wqT[p, c*512 + h*128 + o] = wq[512r + h*128 + o, c*128 + p]
    wqT = nc.dram_tensor("wqT", [128, NDC * HL * HD], BF16,
                         kind="ExternalInput")
    # wk_s = wk * SCALE  (full, natural [kv*128+d, D])
    wk = nc.dram_tensor("wk", [N_KV * HD, DIM], BF16, kind="ExternalInput")
    # wvT[p, c*128 + d] = wv[128r + d, c*128 + p]
    wvT = nc.dram_tensor("wvT", [128, NDC * HD], BF16, kind="ExternalInput")
    # woT[p, h*4096 + jc*128 + j] = wo[jc*128 + j, 512r + h*128 + p]
    woT = nc.dram_tensor("woT", [128, HL * DIM], BF16, kind="ExternalInput")
    # yT[p, b*256 + jc*8 + s] = y_partial[2s+b, jc*128+p]
    yT = nc.dram_tensor("yT", [128, B_LOC * NDC * NC], BF16,
                        kind="ExternalOutput")
    if debug:
        dbg_xq = nc.dram_tensor("dbg_xq", [128, 64], BF16, kind="ExternalOutput")
        dbg_qt = nc.dram_tensor("dbg_qt", [64, DIM], BF16, kind="ExternalOutput")
        dbg_qtT = nc.dram_tensor("dbg_qtT", [128, B_LOC * NDC * NH],
                                 BF16, kind="ExternalOutput")
        dbg_ctx = nc.dram_tensor("dbg_ctx", [NH, B_LOC * DIM], BF16,
                                 kind="ExternalOutput")
        dbg_out = nc.dram_tensor("dbg_out", [NH, B_LOC * HD], BF16,
                                 kind="ExternalOutput")

    rg = [list(range(NC))]
    vs_engines = None  # round-robin copy engines, set below

    with tile.TileContext(nc) as tc:
        with (
            tc.tile_pool(name="persist", bufs=1) as pers,
            tc.tile_pool(name="dram", bufs=1, space="DRAM") as dram,
            tc.tile_pool(name="xpool", bufs=6) as xpool,
            tc.tile_pool(name="xTpool", bufs=3) as xTpool,
            tc.tile_pool(name="attn", bufs=8) as apool,
            tc.tile_pool(name="small", bufs=2) as smallp,
            tc.tile_pool(name="ctxsb", bufs=1) as ctxsbp,
            tc.tile_pool(name="pC", bufs=1) as pC,
            tc.tile_pool(name="pCw", bufs=1) as pCw,
            tc.tile_pool(name="tps", bufs=3, space="PSUM") as tps,
            tc.tile_pool(name="scps", bufs=1, space="PSUM") as scps,
                                    tc.tile_pool(name="ctxps", bufs=1, space="PSUM") as ctxps,
            tc.tile_pool(name="sumps", bufs=1, space="PSUM") as sumps,
            tc.tile_pool(name="miscps", bufs=1, space="PSUM") as miscps,
        ):
            fps = miscps
            pCps = miscps
            yps = miscps
            ident = pers.tile([128, 128], BF16)
            make_identity(nc, ident)
            ones_bf = pers.tile([128, 1], BF16)
            nc.vector.memset(ones_bf, 1.0)

            a2a1_in = dram.tile([BSZ, 512], BF16)
            a2a1_out = dram.tile([BSZ, 512], BF16)
            a2a2_in = [dram.tile([NC * HL, DIM], BF16, name=f"a2a2i{b}")
                       for b in range(B_LOC)]
            a2a2_out = [dram.tile([NC * HL, DIM], BF16, name=f"a2a2o{b}")
                        for b in range(B_LOC)]

            # ---------------- Phase A: xq (head-sharded) -> tiny AllToAll
            qtT_all = pers.tile([128, B_LOC * NDC * NH], BF16, name="qtTall")
            qtT = [qtT_all[:, b * NDC * NH:(b + 1) * NDC * NH]
                   for b in range(B_LOC)]
            stage1 = []
            with (
                tc.tile_pool(name="pA", bufs=1) as pA,
                tc.tile_pool(name="pAw", bufs=16) as pAw,
            ):
                xlT_sb = pA.tile([128, NDC * BSZ], BF16)
                nc.sync.dma_start(out=xlT_sb, in_=xlT[:, :])
                wq_pieces = []
                for q in range(16):
                    wq_q = pAw.tile([128, 2 * HL * HD], BF16, tag="wqq",
                                    name=f"wqq{q}")
                    nc.sync.dma_start(
                        out=wq_q, in_=wqT[:, q * 1024:(q + 1) * 1024])
                    wq_pieces.append(wq_q)
                # xq[b, o] for the local 512-wide o-slice; one PSUM chain
                xq_psf = scps.tile([128, 512], F32, tag="sc")
                xq_ps = xq_psf[0:BSZ]
                for q in range(16):
                    for k in range(2):
                        c = q * 2 + k
                        nc.tensor.matmul(
                            xq_ps,
                            xlT_sb[:, c * BSZ:(c + 1) * BSZ],
                            wq_pieces[q][:, k * 512:(k + 1) * 512],
                            start=(c == 0), stop=(c == NDC - 1))
                xq_sb = pA.tile([BSZ, 512], BF16)
                nc.scalar.copy(out=xq_sb, in_=xq_ps)
                d = nc.scalar.dma_start(out=a2a1_in[:, :], in_=xq_sb)
                stage1.append(d)

            cc1 = None
            if not nocc:
                cc1 = nc.gpsimd.collective_compute(
                    "AllToAll", mybir.AluOpType.bypass,
                    ins=[a2a1_in.opt()], outs=[a2a1_out.opt()],
                    replica_groups=rg)
                for d in stage1:
                    add_dep_helper(cc1.ins, d.ins, reason="a2a1 input ready")

            # qtT[b][p=D, c*32+h] = sum_d xq[2r+b, h*128+d] * wk_s[h*128+d, c*128+p]
            with (
                tc.tile_pool(name="qn", bufs=1) as qn,
                tc.tile_pool(name="wkp", bufs=8) as wkp,
            ):
                # wk pieces per kv group, streamed (SP queue, after wqT)
                wk_dmas = []
                wk_pieces = []
                for kv in range(N_KV):
                    wkq = wkp.tile([HD, DIM], BF16, tag="wkp",
                                   name=f"wk{kv}")
                    dk = nc.sync.dma_start(
                        out=wkq, in_=wk[kv * HD:(kv + 1) * HD, :])

                    wk_dmas.append(dk)
                    wk_pieces.append(wkq)
                xq_loc = qn.tile([B_LOC, DIM], BF16)
                xql_dmas = []
                av = a2a1_out.rearrange("(sq bl) o -> bl sq o", sq=NC)
                for bl in range(B_LOC):
                    d = nc.scalar.dma_start(
                        out=xq_loc[bl:bl + 1].rearrange(
                            "p (sq o) -> p sq o", sq=NC),
                        in_=av[bl])
                    xql_dmas.append(d)
                    if cc1 is not None:
                        add_dep_helper(d.ins, cc1.ins, reason="a2a1 done")
                # xqT2[p=d, 2*hg+bl] via PE transposes
                xqT2_psf = miscps.tile([128, 512], BF16, tag="ctp")
                for c in range(NDC):
                    nc.tensor.transpose(
                        xqT2_psf[:, c * 2:(c + 1) * 2],
                        xq_loc[:, c * 128:(c + 1) * 128],
                        ident[0:B_LOC, 0:B_LOC])
                xqT2_sb = qn.tile([128, NDC * B_LOC], BF16)
                nc.scalar.copy(out=xqT2_sb, in_=xqT2_psf[:, 0:NDC * B_LOC])
                # per kv: qtT chunks [128 D, (c, h, bl)]
                for kv in range(N_KV):
                    if kv % 2 == 0:
                        qt_ps = scps.tile([128, 512], F32, tag="sc")
                    else:
                        qt_ps = miscps.tile([128, 512], F32, tag="ctp")
                    qp = qt_ps.rearrange("p (c h bl) -> p c h bl", c=NDC, h=HL)
                    for c in range(NDC):
                        nc.tensor.matmul(
                            qt_ps[:, c * 8:(c + 1) * 8],
                            wk_pieces[kv][:, c * 128:(c + 1) * 128],
                            xqT2_sb[:, 8 * kv:8 * (kv + 1)],
                            start=True, stop=True)
                    qall = qtT_all.rearrange("p (bl c hh) -> p bl c hh",
                                             bl=B_LOC, c=NDC)
                    if kv % 2 == 0:
                        nc.vector.tensor_copy(
                            out=qall[:, :, :, 4 * kv:4 * (kv + 1)],
                            in_=qt_ps[:, 0:256].rearrange(
                                "p (c h bl) -> p bl c h", c=NDC, h=HL))
                    else:
                        nc.scalar.copy(
                            out=qall[:, :, :, 4 * kv:4 * (kv + 1)],
                            in_=qt_ps[:, 0:256].rearrange(
                                "p (c h bl) -> p bl c h", c=NDC, h=HL))
                if debug:
                    for b in range(B_LOC):
                        nc.sync.dma_start(
                            out=dbg_qtT[:, b * NDC * NH:(b + 1) * NDC * NH],
                            in_=qtT[b])

            # ---------------- Phase B: streaming attention per local batch
            cc2 = [None, None]
            if True:
                xdmas = []
                import os
                _CP = os.environ.get("XTCOPY", "3d1a")
                def xt_copy(g, out, in_):
                    # GPSIMD cannot read PSUM; split PSUM->SBUF copies
                    # between DVE and ACT.
                    if _CP == "alldve":
                        nc.vector.tensor_copy(out=out, in_=in_)
                    elif _CP == "2d2a":
                        if g in (1, 3):
                            nc.scalar.copy(out=out, in_=in_)
                        else:
                            nc.vector.tensor_copy(out=out, in_=in_)
                    elif _CP == "split":
                        if g in (1, 3):
                            nc.scalar.copy(out=out[:, 0:512], in_=in_[:, 0:512])
                            nc.vector.tensor_copy(out=out[:, 512:1024],
                                                  in_=in_[:, 512:1024])
                        else:
                            nc.vector.tensor_copy(out=out, in_=in_)
                    else:
                        if g == 1:
                            nc.scalar.copy(out=out, in_=in_)
                        else:
                            nc.vector.tensor_copy(out=out, in_=in_)
                fin_state = {}
                fin2_state = {}
                fin_stage = {}

                def finalize_part1(bb, ctx_ps_b, sumT_ps_b):
                    sumT_sb = smallp.tile([NH, 1], F32, tag="ssum")
                    nc.vector.tensor_copy(out=sumT_sb, in_=sumT_ps_b)
                    recip = smallp.tile([NH, 1], F32, tag="recip")
                    nc.vector.reciprocal(out=recip, in_=sumT_sb)
                    ctxT_sb = ctxsbp.tile([128, NDC * NH], BF16, tag="ctxT")
                    for half in range(2):
                        nc.scalar.copy(
                            out=ctxT_sb[:, half * 512:(half + 1) * 512],
                            in_=ctx_ps_b[:, half * 512:(half + 1) * 512])
                    fin_state[bb] = (ctxT_sb, recip)

                def finalize_part2_groups(bb, groups, state):
                    last = (bb == B_LOC - 1)
                    ctxT_sb, recip = fin_state[bb]
                    if "ctx_sb0" not in state:
                        state["ctx_sb0"] = ctxsbp.tile([NH, DIM // 2], BF16,
                                                       tag="ctxn0",
                                                       name=f"ctxn0_{bb}")
                        state["ctx_sb1"] = ctxsbp.tile([NH, DIM // 2], BF16,
                                                       tag="ctxn1",
                                                       name=f"ctxn1_{bb}")
                    for g in groups:
                        ctx_sb = (state["ctx_sb0"] if g < 2
                                  else state["ctx_sb1"])
                        goff = (g % 2) * 1024
                        tp2f = tps.tile([128, 1024], BF16, tag="xtp")
                        tp2 = tp2f[0:NH]
                        for k in range(8):
                            c = g * 8 + k
                            nc.tensor.transpose(
                                tp2[:, k * 128:(k + 1) * 128],
                                ctxT_sb[:, c * NH:(c + 1) * NH],
                                ident)
                        if last and g % 2 == 1:
                            nc.scalar.mul(
                                out=ctx_sb[:, goff:goff + 1024],
                                in_=tp2, mul=recip)
                        else:
                            nc.vector.tensor_scalar_mul(
                                ctx_sb[:, goff:goff + 1024], tp2, recip)

                def finalize_part2_stage(bb, state, half):
                    last = (bb == B_LOC - 1)
                    ctx_h = state["ctx_sb0" if half == 0 else "ctx_sb1"]
                    dst = a2a2_in[bb][:, half * 2048:(half + 1) * 2048]
                    if last:
                        d = nc.scalar.dma_start(out=dst, in_=ctx_h)
                    else:
                        d = nc.gpsimd.dma_start(out=dst, in_=ctx_h)
                    state.setdefault("stages", []).append(d)

                def finalize_part2_finish(bb, state):
                    fin_state.pop(bb)
                    fin_stage[bb] = state["stages"][-1]
                    if not nocc:
                        cc2[bb] = nc.gpsimd.collective_compute(
                            "AllToAll", mybir.AluOpType.bypass,
                            ins=[a2a2_in[bb].opt()], outs=[a2a2_out[bb].opt()],
                            replica_groups=rg)
                        for d in state["stages"]:
                            add_dep_helper(cc2[bb].ins, d.ins,
                                           reason="a2a2 input ready")

                for b in range(B_LOC):
                    ctx_ps = ctxps.tile([128, NDC * NH], F32, tag="ctx")
                    sumT_ps = sumps.tile([NH, 1], F32, tag="sumT")

                    def emit_attn(tt, x_sb, xT_sb):
                        sc_full = scps.tile([128, 512], F32, tag="sc")
                        sc_ps = sc_full[:, 0:NH]
                        for c in range(NDC):
                            nc.tensor.matmul(
                                sc_ps,
                                xT_sb[:, c * 128:(c + 1) * 128],
                                qtT[b][:, c * NH:(c + 1) * NH],
                                start=(c == 0), stop=(c == NDC - 1))
                        at_sb = apool.tile([128, NH], BF16, tag="at")
                        nc.scalar.activation(
                            out=at_sb, in_=sc_ps,
                            func=mybir.ActivationFunctionType.Exp)
                        nc.tensor.matmul(sumT_ps, at_sb, ones_bf,
                                         start=(tt == 0), stop=(tt == NT - 1))
                        bank_start = [None, None]
                        for c in range(NDC):
                            mm = nc.tensor.matmul(
                                ctx_ps[:, c * NH:(c + 1) * NH],
                                x_sb[:, c * 128:(c + 1) * 128],
                                at_sb,
                                start=(tt == 0 and c % 16 == 0),
                                stop=(tt == NT - 1),
                                skip_group_check=True)
                            if tt == 0:
                                if c % 16 == 0:
                                    bank_start[c // 16] = mm
                                else:
                                    add_dep_helper(
                                        mm.ins, bank_start[c // 16].ins,
                                        reason="bank wipe first")

                    import os as _os
                    _LAG = int(_os.environ.get("ALAG", "2"))
                    pending = []
                    for tt in range(NT):
                        x_sb = xpool.tile([128, DIM], BF16, tag="x",
                                          name=f"x{b}_{tt}")
                        xd = nc.sync.dma_start(
                            out=x_sb, in_=xp[b, tt * 128:(tt + 1) * 128, :])
                        import os as _os2
                        _XH = _os2.environ.get("XHOLD", "wk")
                        if len(xdmas) == 0 and _XH == "xql" and xql_dmas:
                            add_dep_helper(xd.ins, xql_dmas[-1].ins,
                                           reason="weights+xql first")
                        elif len(xdmas) == 0 and _XH == "wk" and wk_dmas:
                            add_dep_helper(xd.ins, wk_dmas[-1].ins,
                                           reason="wk first")
                        xdmas.append(xd)
                        xT_sb = xTpool.tile([128, DIM], BF16, tag="xT")
                        for g in range(0 if notrans else 4):
                            tp = tps.tile([128, 1024], BF16, tag="xtp")
                            for k in range(8):
                                c = g * 8 + k
                                nc.tensor.transpose(
                                    tp[:, k * 128:(k + 1) * 128],
                                    x_sb[:, c * 128:(c + 1) * 128], ident)
                            xt_copy(g, xT_sb[:, g * 1024:(g + 1) * 1024], tp)
                        if noattn:
                            continue
                        pending.append((tt, x_sb, xT_sb))
                        if len(pending) > _LAG:
                            emit_attn(*pending.pop(0))
                        if b == 1 and 0 in fin_state and tt == int(__import__('os').environ.get('FIN2', '2')):
                            finalize_part2_groups(0, [0, 1], fin2_state)
                            finalize_part2_stage(0, fin2_state, 0)
                            finalize_part2_groups(0, [2, 3], fin2_state)
                            finalize_part2_stage(0, fin2_state, 1)
                            finalize_part2_finish(0, fin2_state)
                    for p in pending:
                        emit_attn(*p)
                    finalize_part1(b, ctx_ps, sumT_ps)
                    if b == B_LOC - 1:
                        st = {}
                        finalize_part2_groups(b, [0, 1], st)
                        finalize_part2_stage(b, st, 0)
                        finalize_part2_groups(b, [2, 3], st)
                        finalize_part2_stage(b, st, 1)
                        finalize_part2_finish(b, st)

                # ---------------- Phase C: output projection per batch slot
                wvT_sb = pCw.tile([128, NDC * HD], BF16)
                dwv = nc.sync.dma_start(out=wvT_sb, in_=wvT[:, :])
                woT_sb = pCw.tile([128, HL * DIM], BF16)
                dwo = nc.sync.dma_start(out=woT_sb, in_=woT[:, :])
                import os as _os3
                _WVM = _os3.environ.get("WVMODE", "mid")
                if _WVM == "tail" and (B_LOC - 1) in fin_stage:
                    add_dep_helper(dwv.ins, fin_stage[B_LOC - 1].ins,
                                   reason="wv in collective window")
                    add_dep_helper(dwo.ins, fin_stage[B_LOC - 1].ins,
                                   reason="wo in collective window")
                else:
                    _WVK = int(_os3.environ.get("WVK", str(NT + 4)))
                    add_dep_helper(dwv.ins, xdmas[_WVK].ins,
                                   reason="late wv")
                    add_dep_helper(dwo.ins, xdmas[_WVK + 4].ins,
                                   reason="late wo")
                yT_sb = pCw.tile([128, NDC * B_LOC * NC], BF16)
                for b in range(B_LOC):
                    ctxgf = xpool.tile([128, DIM], BF16, tag="x",
                                       name=f"ctxg{b}")
                    ctxg = ctxgf[0:NH]
                    d = nc.scalar.dma_start(out=ctxg, in_=a2a2_out[b][:, :])
                    if cc2[b] is not None:
                        add_dep_helper(d.ins, cc2[b].ins,
                                       reason="a2a2 done")
                    ctxgT = pC.tile([128, NDC * NH], BF16, tag="ctxgT")
                    for hf in range(2):
                        tpg = tps.tile([128, 1024], BF16, tag="xtp",
                                       name=f"tpg{b}_{hf}")
                        for k in range(16):
                            c = hf * 16 + k
                            nc.tensor.transpose(
                                tpg[:, k * 32:(k + 1) * 32],
                                ctxg[:, c * 128:(c + 1) * 128],
                                ident[0:NH, 0:NH])
                        nc.vector.tensor_copy(
                            out=ctxgT[:, hf * 512:(hf + 1) * 512],
                            in_=tpg[:, 0:512])
                    # outT[d, (s,h)] = sum_D wvT[D, d]^T ctxgT[D, (s,h)]
                    op_ps = pCps.tile([HD, NH], F32, tag="ctp")
                    for c in range(NDC):
                        nc.tensor.matmul(op_ps,
                                         wvT_sb[:, c * 128:(c + 1) * 128],
                                         ctxgT[:, c * NH:(c + 1) * NH],
                                         start=(c == 0), stop=(c == NDC - 1))
                    outT = pC.tile([128, NH], BF16, tag="outT")
                    nc.vector.tensor_copy(out=outT[0:HD], in_=op_ps)
                    # yT[j, s] = sum_h sum_d woT[d, (h, jc, j)] * outT[d, (s, h)]
                    ov = outT.rearrange("p (s h) -> p h s", h=HL)
                    y_ps = yps.tile([128, NDC * NC], F32, tag="ctp")
                    for jc in range(NDC):
                        for h in range(HL):
                            nc.tensor.matmul(
                                y_ps[:, jc * NC:(jc + 1) * NC],
                                woT_sb[:, h * DIM + jc * 128:
                                       h * DIM + (jc + 1) * 128],
                                ov[:, h, :],
                                start=(h == 0), stop=(h == HL - 1))
                    yv = yT_sb.rearrange("p (b jcs) -> b p jcs", b=B_LOC)
                    nc.vector.tensor_copy(out=yv[b], in_=y_ps)
                    nc.sync.dma_start(
                        out=yT.rearrange("p (b jcs) -> b p jcs", b=B_LOC)[b],
                        in_=yv[b])

    nc.finalize()
    return nc


_PROGRAM_CACHE = {}


def _prep_inputs(x_pre, wq, wk, wv, wo):
    """Shard + cast + pre-transpose on host. Returns in_maps for 8 cores."""
    xlT_full = np.ascontiguousarray(
        x_pre[:, -1, :].T.astype(NPBF))                    # [4096, 16]
    xlT_full = xlT_full.reshape(NDC, 128, BSZ).transpose(1, 0, 2)  # [128,c,b]
    xlT_flat = np.ascontiguousarray(xlT_full.reshape(128, NDC * BSZ))

    wk_s = (wk * SCALE).astype(NPBF)
    in_maps = []
    for r in range(NC):
        # wqT[p, c, h, o] = wq[512r + h*128 + o, c*128 + p]
        wq_sl = wq[512 * r:512 * (r + 1), :].astype(NPBF)   # [512, 4096]
        wqT_r = wq_sl.reshape(HL, 128, NDC, 128).transpose(3, 2, 0, 1)
        wqT_r = np.ascontiguousarray(wqT_r.reshape(128, NDC * HL * HD))
        # wvT[p, c, d] = wv[128r + d, c*128 + p]
        wv_sl = wv[128 * r:128 * (r + 1), :].astype(NPBF)   # [128 d, 4096 D]
        wvT_r = wv_sl.reshape(128, NDC, 128).transpose(2, 1, 0)
        wvT_r = np.ascontiguousarray(wvT_r.reshape(128, NDC * HD))
        # woT[p, h, jc, j] = wo[jc*128 + j, 512r + h*128 + p]
        wo_sl = wo[:, 512 * r:512 * (r + 1)].astype(NPBF)   # [4096 j, 512 o]
        woT_r = wo_sl.reshape(NDC, 128, HL, 128).transpose(3, 2, 0, 1)
        woT_r = np.ascontiguousarray(woT_r.reshape(128, HL * DIM))
        in_maps.append({
            "xp": np.ascontiguousarray(x_pre[2 * r:2 * r + 2].astype(NPBF)),
            "xlT": xlT_flat,
            "wqT": wqT_r,
            "wk": np.ascontiguousarray(wk_s),
            "wvT": wvT_r,
            "woT": woT_r,
        })
    return in_maps


def kernel(x_pre, wq, wk, wv, wo, _trace=False, _tmpdir=None, _debug=False):
    x_pre = np.asarray(x_pre, dtype=np.float32)
    wq = np.asarray(wq, dtype=np.float32)
    wk = np.asarray(wk, dtype=np.float32)
    wv = np.asarray(wv, dtype=np.float32)
    wo = np.asarray(wo, dtype=np.float32)

    key = "nc_dbg" if _debug else "nc"
    if key not in _PROGRAM_CACHE:
        _PROGRAM_CACHE[key] = build_program(debug=_debug)
        _PROGRAM_CACHE["nc"] = _PROGRAM_CACHE[key]
    nc = _PROGRAM_CACHE[key]

    in_maps = _prep_inputs(x_pre, wq, wk, wv, wo)

    kwargs = {}
    if _trace:
        kwargs = dict(trace=True, trace_cores=[0])
    if _tmpdir is not None:
        kwargs["tmpdir"] = _tmpdir
    res = run_bass_kernel_spmd(nc, in_maps, core_ids=list(range(NC)), **kwargs)

    y = np.zeros((BSZ, DIM), np.float64)
    for r in range(NC):
        yT_r = np.asarray(res.results[r]["yT"], np.float32)
        yT_r = yT_r.reshape(128, B_LOC, NDC, NC)
        # y[2s+b, jc*128+p] += yT_r[p, b, jc, s]
        y += yT_r.transpose(3, 1, 2, 0).reshape(BSZ, DIM)
    if _debug:
        _PROGRAM_CACHE["dbg"] = res
    if _trace:
        print("HW exec time:", res.exec_time_ns, "ns")
    return y.astype(np.float32).reshape(BSZ, 1, DIM)



# revision 65
# speedup vs baseline: 1.0289x; 1.0102x over previous
"""Bass/Trainium2 kernel for GQA decode attention (fused K-projection form).

Reference computation:
  x = x_pre[:, -1, :]                               # [16, 4096]
  xq = (x @ wq.T) -> [b, 32, 128]
  qt[b,h,:] = xq[b,h,:] @ wk[kv(h)*128:+128, :]     # [b, 32, 4096]
  scores = qt . x_pre / sqrt(128)                   # [b, 32, 2048]
  attn = softmax_t(scores)
  ctx[b,h,:] = sum_t attn[b,h,t] * x_pre[b,t,:]     # [b, 32, 4096]  (lazy-V)
  out[b,h,d] = sum_D ctx[b,h,D] * wv[kv(h)*128+d,D] # [b, 32, 128]
  y = out.flat @ wo.T                               # [16, 4096]

Sharding (8 cores): batch-parallel attention (2 batches/core) +
head-parallel projections (4 heads = 1 kv group/core), exchanged with
AllToAll collectives.  All device data is bf16 (f32 PSUM accumulation);
weights are pre-transposed on the host into the layouts the PE consumes,
and the big matmuls are arranged stationary-heavy (large lhsT, narrow
moving operand) so PE streaming cost is minimized.
"""

import math

import numpy as np
import ml_dtypes

import concourse.bass as bass
import concourse.mybir as mybir
import concourse.tile as tile
from concourse import bacc
from concourse.bass_utils import run_bass_kernel_spmd
from concourse.masks import make_identity
from concourse.tile import add_dep_helper

F32 = mybir.dt.float32
BF16 = mybir.dt.bfloat16
F8 = mybir.dt.float8e4
NPBF = ml_dtypes.bfloat16
NPF8 = ml_dtypes.float8_e4m3
WK_PRESCALE = 1024.0
WQ_PRESCALE = 64.0

NC = 8
BSZ = 16
SEQ = 2048
DIM = 4096
NH = 32
HD = 128
B_LOC = 2        # batches per core
HL = 4           # local heads per core (= one kv group)
N_KV = 8
NT = SEQ // 128  # 16 t-tiles per batch
NDC = DIM // 128 # 32 D-chunks
SCALE = 1.0 / math.sqrt(HD)


def build_program(debug=False, nocc=False, noattn=False, notrans=False, nocopy=False):
    nc = bacc.Bacc("TRN2", target_bir_lowering=False, debug=False)

    xp = nc.dram_tensor("xp", [B_LOC, SEQ, DIM], BF16, kind="ExternalInput")
    # xlT[p, c, b] = x_pre[b, -1, c*128+p]
    xlT = nc.dram_tensor("xlT", [128, NDC * BSZ], BF16, kind="ExternalInput")
    # wqT[p, c*512 + h*128 + o] = wq[512r + h*128 + o, c*128 + p]
    wqT = nc.dram_tensor("wqT", [128, NDC * HL * HD], BF16,
                         kind="ExternalInput")
    # wk_s = wk * SCALE  (full, natural [kv*128+d, D])
    wk = nc.dram_tensor("wk", [N_KV * HD, DIM], BF16, kind="ExternalInput")
    # wvT[p, c*128 + d] = wv[128r + d, c*128 + p]
    wvT = nc.dram_tensor("wvT", [128, NDC * HD], BF16, kind="ExternalInput")
    # woT[p, h*4096 + jc*128 + j] = wo[jc*128 + j, 512r + h*128 + p]
    woT = nc.dram_tensor("woT", [128, HL * DIM], BF16, kind="ExternalInput")
    # yT[p, b*256 + jc*8 + s] = y_partial[2s+b, jc*128+p]
    yT = nc.dram_tensor("yT", [128, B_LOC * NDC * NC], BF16,
                        kind="ExternalOutput")
    if debug:
        dbg_xq = nc.dram_tensor("dbg_xq", [128, 64], BF16, kind="ExternalOutput")
        dbg_qt = nc.dram_tensor("dbg_qt", [64, DIM], BF16, kind="ExternalOutput")
        dbg_qtT = nc.dram_tensor("dbg_qtT", [128, B_LOC * NDC * NH],
                                 BF16, kind="ExternalOutput")
        dbg_ctx = nc.dram_tensor("dbg_ctx", [NH, B_LOC * DIM], BF16,
                                 kind="ExternalOutput")
        dbg_out = nc.dram_tensor("dbg_out", [NH, B_LOC * HD], BF16,
                                 kind="ExternalOutput")

    rg = [list(range(NC))]
    vs_engines = None  # round-robin copy engines, set below

    with tile.TileContext(nc) as tc:
        with (
            tc.tile_pool(name="persist", bufs=1) as pers,
            tc.tile_pool(name="dram", bufs=1, space="DRAM") as dram,
            tc.tile_pool(name="xpool", bufs=6) as xpool,
            tc.tile_pool(name="xTpool", bufs=3) as xTpool,
            tc.tile_pool(name="attn", bufs=8) as apool,
            tc.tile_pool(name="small", bufs=2) as smallp,
            tc.tile_pool(name="ctxsb", bufs=1) as ctxsbp,
            tc.tile_pool(name="pC", bufs=1) as pC,
            tc.tile_pool(name="pCw", bufs=1) as pCw,
            tc.tile_pool(name="tps", bufs=3, space="PSUM") as tps,
            tc.tile_pool(name="scps", bufs=1, space="PSUM") as scps,
                                    tc.tile_pool(name="ctxps", bufs=1, space="PSUM") as ctxps,
            tc.tile_pool(name="sumps", bufs=1, space="PSUM") as sumps,
            tc.tile_pool(name="miscps", bufs=1, space="PSUM") as miscps,
        ):
            fps = miscps
            pCps = miscps
            yps = miscps
            ident = pers.tile([128, 128], BF16)
            make_identity(nc, ident)
            ones_bf = pers.tile([128, 1], BF16)
            nc.vector.memset(ones_bf, 1.0)

            a2a1_in = dram.tile([BSZ, 512], BF16)
            a2a1_out = dram.tile([BSZ, 512], BF16)
            a2a2_in = [dram.tile([NC * HL, DIM], BF16, name=f"a2a2i{b}")
                       for b in range(B_LOC)]
            a2a2_out = [dram.tile([NC * HL, DIM], BF16, name=f"a2a2o{b}")
                        for b in range(B_LOC)]

            # ---------------- Phase A: xq (head-sharded) -> tiny AllToAll
            qtT_all = pers.tile([128, B_LOC * NDC * NH], BF16, name="qtTall")
            qtT = [qtT_all[:, b * NDC * NH:(b + 1) * NDC * NH]
                   for b in range(B_LOC)]
            stage1 = []
            with (
                tc.tile_pool(name="pA", bufs=1) as pA,
                tc.tile_pool(name="pAw", bufs=16) as pAw,
            ):
                xlT_sb = pA.tile([128, NDC * BSZ], BF16)
                nc.sync.dma_start(out=xlT_sb, in_=xlT[:, :])
                wq_pieces = []
                for q in range(16):
                    wq_q = pAw.tile([128, 2 * HL * HD], BF16, tag="wqq",
                                    name=f"wqq{q}")
                    nc.sync.dma_start(
                        out=wq_q, in_=wqT[:, q * 1024:(q + 1) * 1024])
                    wq_pieces.append(wq_q)
                # xq[b, o] for the local 512-wide o-slice; one PSUM chain
                xq_psf = scps.tile([128, 512], F32, tag="sc")
                xq_ps = xq_psf[0:BSZ]
                for q in range(16):
                    for k in range(2):
                        c = q * 2 + k
                        nc.tensor.matmul(
                            xq_ps,
                            xlT_sb[:, c * BSZ:(c + 1) * BSZ],
                            wq_pieces[q][:, k * 512:(k + 1) * 512],
                            start=(c == 0), stop=(c == NDC - 1))
                xq_sb = pA.tile([BSZ, 512], BF16)
                nc.scalar.copy(out=xq_sb, in_=xq_ps)
                d = nc.scalar.dma_start(out=a2a1_in[:, :], in_=xq_sb)
                stage1.append(d)

            cc1 = None
            if not nocc:
                cc1 = nc.gpsimd.collective_compute(
                    "AllToAll", mybir.AluOpType.bypass,
                    ins=[a2a1_in.opt()], outs=[a2a1_out.opt()],
                    replica_groups=rg)
                for d in stage1:
                    add_dep_helper(cc1.ins, d.ins, reason="a2a1 input ready")

            # qtT[b][p=D, c*32+h] = sum_d xq[2r+b, h*128+d] * wk_s[h*128+d, c*128+p]
            with (
                tc.tile_pool(name="qn", bufs=1) as qn,
                tc.tile_pool(name="wkp", bufs=8) as wkp,
            ):
                # wk pieces per kv group, streamed (SP queue, after wqT)
                wk_dmas = []
                wk_pieces = []
                for kv in range(N_KV):
                    wkq = wkp.tile([HD, DIM], BF16, tag="wkp",
                                   name=f"wk{kv}")
                    dk = nc.sync.dma_start(
                        out=wkq, in_=wk[kv * HD:(kv + 1) * HD, :])

                    wk_dmas.append(dk)
                    wk_pieces.append(wkq)
                xq_loc = qn.tile([B_LOC, DIM], BF16)
                xql_dmas = []
                av = a2a1_out.rearrange("(sq bl) o -> bl sq o", sq=NC)
                for bl in range(B_LOC):
                    d = nc.scalar.dma_start(
                        out=xq_loc[bl:bl + 1].rearrange(
                            "p (sq o) -> p sq o", sq=NC),
                        in_=av[bl])
                    xql_dmas.append(d)
                    if cc1 is not None:
                        add_dep_helper(d.ins, cc1.ins, reason="a2a1 done")
                # xqT2[p=d, 2*hg+bl] via PE transposes
                xqT2_psf = miscps.tile([128, 512], BF16, tag="ctp")
                for c in range(NDC):
                    nc.tensor.transpose(
                        xqT2_psf[:, c * 2:(c + 1) * 2],
                        xq_loc[:, c * 128:(c + 1) * 128],
                        ident[0:B_LOC, 0:B_LOC])
                xqT2_sb = qn.tile([128, NDC * B_LOC], BF16)
                nc.scalar.copy(out=xqT2_sb, in_=xqT2_psf[:, 0:NDC * B_LOC])
                # per kv: qtT chunks [128 D, (c, h, bl)]
                for kv in range(N_KV):
                    if kv % 2 == 0:
                        qt_ps = scps.tile([128, 512], F32, tag="sc")
                    else:
                        qt_ps = miscps.tile([128, 512], F32, tag="ctp")
                    qp = qt_ps.rearrange("p (c h bl) -> p c h bl", c=NDC, h=HL)
                    for c in range(NDC):
                        nc.tensor.matmul(
                            qt_ps[:, c * 8:(c + 1) * 8],
                            wk_pieces[kv][:, c * 128:(c + 1) * 128],
                            xqT2_sb[:, 8 * kv:8 * (kv + 1)],
                            start=True, stop=True)
                    qall = qtT_all.rearrange("p (bl c hh) -> p bl c hh",
                                             bl=B_LOC, c=NDC)
                    if kv % 2 == 0:
                        nc.vector.tensor_copy(
                            out=qall[:, :, :, 4 * kv:4 * (kv + 1)],
                            in_=qt_ps[:, 0:256].rearrange(
                                "p (c h bl) -> p bl c h", c=NDC, h=HL))
                    else:
                        nc.scalar.copy(
                            out=qall[:, :, :, 4 * kv:4 * (kv + 1)],
                            in_=qt_ps[:, 0:256].rearrange(
                                "p (c h bl) -> p bl c h", c=NDC, h=HL))
                if debug:
                    for b in range(B_LOC):
                        nc.sync.dma_start(
                            out=dbg_qtT[:, b * NDC * NH:(b + 1) * NDC * NH],
                            in_=qtT[b])

            # ---------------- Phase B: streaming attention per local batch
            cc2 = [None, None]
            if True:
                xdmas = []
                import os
                _CP = os.environ.get("XTCOPY", "3d1a")
                def xt_copy(g, out, in_):
                    # GPSIMD cannot read PSUM; split PSUM->SBUF copies
                    # between DVE and ACT.
                    if _CP == "alldve":
                        nc.vector.tensor_copy(out=out, in_=in_)
                    elif _CP == "2d2a":
                        if g in (1, 3):
                            nc.scalar.copy(out=out, in_=in_)
                        else:
                            nc.vector.tensor_copy(out=out, in_=in_)
                    elif _CP == "split":
                        if g in (1, 3):
                            nc.scalar.copy(out=out[:, 0:512], in_=in_[:, 0:512])
                            nc.vector.tensor_copy(out=out[:, 512:1024],
                                                  in_=in_[:, 512:1024])
                        else:
                            nc.vector.tensor_copy(out=out, in_=in_)
                    else:
                        if g == 1:
                            nc.scalar.copy(out=out, in_=in_)
                        else:
                            nc.vector.tensor_copy(out=out, in_=in_)
                fin_state = {}
                fin2_state = {}
                fin_stage = {}

                def finalize_part1(bb, ctx_ps_b, sumT_ps_b):
                    sumT_sb = smallp.tile([NH, 1], F32, tag="ssum")
                    nc.vector.tensor_copy(out=sumT_sb, in_=sumT_ps_b)
                    recip = smallp.tile([NH, 1], F32, tag="recip")
                    nc.vector.reciprocal(out=recip, in_=sumT_sb)
                    ctxT_sb = ctxsbp.tile([128, NDC * NH], BF16, tag="ctxT")
                    for half in range(2):
                        nc.scalar.copy(
                            out=ctxT_sb[:, half * 512:(half + 1) * 512],
                            in_=ctx_ps_b[:, half * 512:(half + 1) * 512])
                    fin_state[bb] = (ctxT_sb, recip)

                def finalize_part2_groups(bb, groups, state):
                    last = (bb == B_LOC - 1)
                    ctxT_sb, recip = fin_state[bb]
                    if "ctx_sb0" not in state:
                        state["ctx_sb0"] = ctxsbp.tile([NH, DIM // 2], BF16,
                                                       tag="ctxn0",
                                                       name=f"ctxn0_{bb}")
                        state["ctx_sb1"] = ctxsbp.tile([NH, DIM // 2], BF16,
                                                       tag="ctxn1",
                                                       name=f"ctxn1_{bb}")
                    for g in groups:
                        ctx_sb = (state["ctx_sb0"] if g < 2
                                  else state["ctx_sb1"])
                        goff = (g % 2) * 1024
                        tp2f = tps.tile([128, 1024], BF16, tag="xtp")
                        tp2 = tp2f[0:NH]
                        for k in range(8):
                            c = g * 8 + k
                            nc.tensor.transpose(
                                tp2[:, k * 128:(k + 1) * 128],
                                ctxT_sb[:, c * NH:(c + 1) * NH],
                                ident)
                        if last and g == 1:
                            nc.scalar.mul(
                                out=ctx_sb[:, goff:goff + 1024],
                                in_=tp2, mul=recip)
                        else:
                            nc.vector.tensor_scalar_mul(
                                ctx_sb[:, goff:goff + 1024], tp2, recip)

                def finalize_part2_stage(bb, state, half):
                    last = (bb == B_LOC - 1)
                    ctx_h = state["ctx_sb0" if half == 0 else "ctx_sb1"]
                    dst = a2a2_in[bb][:, half * 2048:(half + 1) * 2048]
                    if last:
                        d = nc.scalar.dma_start(out=dst, in_=ctx_h)
                    else:
                        d = nc.gpsimd.dma_start(out=dst, in_=ctx_h)
                    state.setdefault("stages", []).append(d)

                def finalize_part2_finish(bb, state):
                    fin_state.pop(bb)
                    fin_stage[bb] = state["stages"][-1]
                    if not nocc:
                        cc2[bb] = nc.gpsimd.collective_compute(
                            "AllToAll", mybir.AluOpType.bypass,
                            ins=[a2a2_in[bb].opt()], outs=[a2a2_out[bb].opt()],
                            replica_groups=rg)
                        for d in state["stages"]:
                            add_dep_helper(cc2[bb].ins, d.ins,
                                           reason="a2a2 input ready")

                for b in range(B_LOC):
                    ctx_ps = ctxps.tile([128, NDC * NH], F32, tag="ctx")
                    sumT_ps = sumps.tile([NH, 1], F32, tag="sumT")

                    def emit_attn(tt, x_sb, xT_sb):
                        # alternate scores PSUM between scps and the
                        # attention-idle miscps bank: free double-buffering
                        # that unserializes scores(t+1) from exp(t)'s read.
                        if tt % 2 == 0:
                            sc_full = scps.tile([128, 512], F32, tag="sc")
                        else:
                            sc_full = miscps.tile([128, 512], F32,
                                                  tag="ctp")
                        sc_ps = sc_full[:, 0:NH]
                        for c in range(NDC):
                            nc.tensor.matmul(
                                sc_ps,
                                xT_sb[:, c * 128:(c + 1) * 128],
                                qtT[b][:, c * NH:(c + 1) * NH],
                                start=(c == 0), stop=(c == NDC - 1))
                        at_sb = apool.tile([128, NH], BF16, tag="at")
                        nc.scalar.activation(
                            out=at_sb, in_=sc_ps,
                            func=mybir.ActivationFunctionType.Exp)
                        nc.tensor.matmul(sumT_ps, at_sb, ones_bf,
                                         start=(tt == 0), stop=(tt == NT - 1))
                        bank_start = [None, None]
                        for c in range(NDC):
                            mm = nc.tensor.matmul(
                                ctx_ps[:, c * NH:(c + 1) * NH],
                                x_sb[:, c * 128:(c + 1) * 128],
                                at_sb,
                                start=(tt == 0 and c % 16 == 0),
                                stop=(tt == NT - 1),
                                skip_group_check=True)
                            if tt == 0:
                                if c % 16 == 0:
                                    bank_start[c // 16] = mm
                                else:
                                    add_dep_helper(
                                        mm.ins, bank_start[c // 16].ins,
                                        reason="bank wipe first")

                    import os as _os
                    _LAG = int(_os.environ.get("ALAG", "2"))
                    pending = []
                    for tt in range(NT):
                        x_sb = xpool.tile([128, DIM], BF16, tag="x",
                                          name=f"x{b}_{tt}")
                        xd = nc.sync.dma_start(
                            out=x_sb, in_=xp[b, tt * 128:(tt + 1) * 128, :])
                        import os as _os2
                        _XH = _os2.environ.get("XHOLD", "wk")
                        if len(xdmas) == 0 and _XH == "xql" and xql_dmas:
                            add_dep_helper(xd.ins, xql_dmas[-1].ins,
                                           reason="weights+xql first")
                        elif len(xdmas) == 0 and _XH == "wk" and wk_dmas:
                            add_dep_helper(xd.ins, wk_dmas[-1].ins,
                                           reason="wk first")
                        xdmas.append(xd)
                        xT_sb = xTpool.tile([128, DIM], BF16, tag="xT")
                        for g in range(0 if notrans else 4):
                            tp = tps.tile([128, 1024], BF16, tag="xtp")
                            for k in range(8):
                                c = g * 8 + k
                                nc.tensor.transpose(
                                    tp[:, k * 128:(k + 1) * 128],
                                    x_sb[:, c * 128:(c + 1) * 128], ident)
                            xt_copy(g, xT_sb[:, g * 1024:(g + 1) * 1024], tp)
                        if noattn:
                            continue
                        pending.append((tt, x_sb, xT_sb))
                        if len(pending) > _LAG:
                            emit_attn(*pending.pop(0))
                        if b == 1 and 0 in fin_state and tt == int(__import__('os').environ.get('FIN2', '2')):
                            finalize_part2_groups(0, [0, 1], fin2_state)
                            finalize_part2_stage(0, fin2_state, 0)
                            finalize_part2_groups(0, [2, 3], fin2_state)
                            finalize_part2_stage(0, fin2_state, 1)
                            finalize_part2_finish(0, fin2_state)
                    for p in pending:
                        emit_attn(*p)
                    finalize_part1(b, ctx_ps, sumT_ps)
                    if b == B_LOC - 1:
                        st = {}
                        finalize_part2_groups(b, [0, 1], st)
                        finalize_part2_stage(b, st, 0)
                        finalize_part2_groups(b, [2, 3], st)
                        finalize_part2_stage(b, st, 1)
                        finalize_part2_finish(b, st)

                # ---------------- Phase C: output projection per batch slot
                wvT_sb = pCw.tile([128, NDC * HD], BF16)
                dwv = nc.sync.dma_start(out=wvT_sb, in_=wvT[:, :])
                woT_sb = pCw.tile([128, HL * DIM], BF16)
                dwo = nc.sync.dma_start(out=woT_sb, in_=woT[:, :])
                import os as _os3
                _WVM = _os3.environ.get("WVMODE", "mid")
                if _WVM == "tail" and (B_LOC - 1) in fin_stage:
                    add_dep_helper(dwv.ins, fin_stage[B_LOC - 1].ins,
                                   reason="wv in collective window")
                    add_dep_helper(dwo.ins, fin_stage[B_LOC - 1].ins,
                                   reason="wo in collective window")
                else:
                    _WVK = int(_os3.environ.get("WVK", str(NT + 4)))
                    add_dep_helper(dwv.ins, xdmas[_WVK].ins,
                                   reason="late wv")
                    add_dep_helper(dwo.ins, xdmas[_WVK + 4].ins,
                                   reason="late wo")
                yT_sb = pCw.tile([128, NDC * B_LOC * NC], BF16)
                for b in range(B_LOC):
                    ctxgf = xpool.tile([128, DIM], BF16, tag="x",
                                       name=f"ctxg{b}")
                    ctxg = ctxgf[0:NH]
                    d = nc.scalar.dma_start(out=ctxg, in_=a2a2_out[b][:, :])
                    if cc2[b] is not None:
                        add_dep_helper(d.ins, cc2[b].ins,
                                       reason="a2a2 done")
                    ctxgT = pC.tile([128, NDC * NH], BF16, tag="ctxgT")
                    for hf in range(2):
                        tpg = tps.tile([128, 1024], BF16, tag="xtp",
                                       name=f"tpg{b}_{hf}")
                        for k in range(16):
                            c = hf * 16 + k
                            nc.tensor.transpose(
                                tpg[:, k * 32:(k + 1) * 32],
                                ctxg[:, c * 128:(c + 1) * 128],
                                ident[0:NH, 0:NH])
                        nc.vector.tensor_copy(
                            out=ctxgT[:, hf * 512:(hf + 1) * 512],
                            in_=tpg[:, 0:512])
                    # outT[d, (s,h)] = sum_D wvT[D, d]^T ctxgT[D, (s,h)]
                    op_ps = pCps.tile([HD, NH], F32, tag="ctp")
                    for c in range(NDC):
                        nc.tensor.matmul(op_ps,
                                         wvT_sb[:, c * 128:(c + 1) * 128],
                                         ctxgT[:, c * NH:(c + 1) * NH],
                                         start=(c == 0), stop=(c == NDC - 1))
                    outT = pC.tile([128, NH], BF16, tag="outT")
                    nc.vector.tensor_copy(out=outT[0:HD], in_=op_ps)
                    # yT[j, s] = sum_h sum_d woT[d, (h, jc, j)] * outT[d, (s, h)]
                    ov = outT.rearrange("p (s h) -> p h s", h=HL)
                    y_ps = yps.tile([128, NDC * NC], F32, tag="ctp")
                    for jc in range(NDC):
                        for h in range(HL):
                            nc.tensor.matmul(
                                y_ps[:, jc * NC:(jc + 1) * NC],
                                woT_sb[:, h * DIM + jc * 128:
                                       h * DIM + (jc + 1) * 128],
                                ov[:, h, :],
                                start=(h == 0), stop=(h == HL - 1))
                    yv = yT_sb.rearrange("p (b jcs) -> b p jcs", b=B_LOC)
                    nc.vector.tensor_copy(out=yv[b], in_=y_ps)
                    nc.sync.dma_start(
                        out=yT.rearrange("p (b jcs) -> b p jcs", b=B_LOC)[b],
                        in_=yv[b])

    nc.finalize()
    return nc


_PROGRAM_CACHE = {}


def _prep_inputs(x_pre, wq, wk, wv, wo):
    """Shard + cast + pre-transpose on host. Returns in_maps for 8 cores."""
    xlT_full = np.ascontiguousarray(
        x_pre[:, -1, :].T.astype(NPBF))                    # [4096, 16]
    xlT_full = xlT_full.reshape(NDC, 128, BSZ).transpose(1, 0, 2)  # [128,c,b]
    xlT_flat = np.ascontiguousarray(xlT_full.reshape(128, NDC * BSZ))

    wk_s = (wk * SCALE).astype(NPBF)
    in_maps = []
    for r in range(NC):
        # wqT[p, c, h, o] = wq[512r + h*128 + o, c*128 + p]
        wq_sl = wq[512 * r:512 * (r + 1), :].astype(NPBF)   # [512, 4096]
        wqT_r = wq_sl.reshape(HL, 128, NDC, 128).transpose(3, 2, 0, 1)
        wqT_r = np.ascontiguousarray(wqT_r.reshape(128, NDC * HL * HD))
        # wvT[p, c, d] = wv[128r + d, c*128 + p]
        wv_sl = wv[128 * r:128 * (r + 1), :].astype(NPBF)   # [128 d, 4096 D]
        wvT_r = wv_sl.reshape(128, NDC, 128).transpose(2, 1, 0)
        wvT_r = np.ascontiguousarray(wvT_r.reshape(128, NDC * HD))
        # woT[p, h, jc, j] = wo[jc*128 + j, 512r + h*128 + p]
        wo_sl = wo[:, 512 * r:512 * (r + 1)].astype(NPBF)   # [4096 j, 512 o]
        woT_r = wo_sl.reshape(NDC, 128, HL, 128).transpose(3, 2, 0, 1)
        woT_r = np.ascontiguousarray(woT_r.reshape(128, HL * DIM))
        in_maps.append({
            "xp": np.ascontiguousarray(x_pre[2 * r:2 * r + 2].astype(NPBF)),
            "xlT": xlT_flat,
            "wqT": wqT_r,
            "wk": np.ascontiguousarray(wk_s),
            "wvT": wvT_r,
            "woT": woT_r,
        })
    return in_maps


def kernel(x_pre, wq, wk, wv, wo, _trace=False, _tmpdir=None, _debug=False):
    x_pre = np.asarray(x_pre, dtype=np.float32)
    wq = np.asarray(wq, dtype=np.float32)
    wk = np.asarray(wk, dtype=np.float32)
    wv = np.asarray(wv, dtype=np.float32)
    wo = np.asarray(wo, dtype=np.float32)

    key = "nc_dbg" if _debug else "nc"
    if key not in _PROGRAM_CACHE:
        _PROGRAM_CACHE[key] = build_program(debug=_debug)
        _PROGRAM_CACHE["nc"] = _PROGRAM_CACHE[key]
    nc = _PROGRAM_CACHE[key]

    in_maps = _prep_inputs(x_pre, wq, wk, wv, wo)

    kwargs = {}
    if _trace:
        kwargs = dict(trace=True, trace_cores=[0])
    if _tmpdir is not None:
        kwargs["tmpdir"] = _tmpdir
    res = run_bass_kernel_spmd(nc, in_maps, core_ids=list(range(NC)), **kwargs)

    y = np.zeros((BSZ, DIM), np.float64)
    for r in range(NC):
        yT_r = np.asarray(res.results[r]["yT"], np.float32)
        yT_r = yT_r.reshape(128, B_LOC, NDC, NC)
        # y[2s+b, jc*128+p] += yT_r[p, b, jc, s]
        y += yT_r.transpose(3, 1, 2, 0).reshape(BSZ, DIM)
    if _debug:
        _PROGRAM_CACHE["dbg"] = res
    if _trace:
        print("HW exec time:", res.exec_time_ns, "ns")
    return y.astype(np.float32).reshape(BSZ, 1, DIM)



# revision 66
# speedup vs baseline: 1.0291x; 1.0001x over previous
"""Bass/Trainium2 kernel for GQA decode attention (fused K-projection form).

Reference computation:
  x = x_pre[:, -1, :]                               # [16, 4096]
  xq = (x @ wq.T) -> [b, 32, 128]
  qt[b,h,:] = xq[b,h,:] @ wk[kv(h)*128:+128, :]     # [b, 32, 4096]
  scores = qt . x_pre / sqrt(128)                   # [b, 32, 2048]
  attn = softmax_t(scores)
  ctx[b,h,:] = sum_t attn[b,h,t] * x_pre[b,t,:]     # [b, 32, 4096]  (lazy-V)
  out[b,h,d] = sum_D ctx[b,h,D] * wv[kv(h)*128+d,D] # [b, 32, 128]
  y = out.flat @ wo.T                               # [16, 4096]

Sharding (8 cores): batch-parallel attention (2 batches/core) +
head-parallel projections (4 heads = 1 kv group/core), exchanged with
AllToAll collectives.  All device data is bf16 (f32 PSUM accumulation);
weights are pre-transposed on the host into the layouts the PE consumes,
and the big matmuls are arranged stationary-heavy (large lhsT, narrow
moving operand) so PE streaming cost is minimized.
"""

import math

import numpy as np
import ml_dtypes

import concourse.bass as bass
import concourse.mybir as mybir
import concourse.tile as tile
from concourse import bacc
from concourse.bass_utils import run_bass_kernel_spmd
from concourse.masks import make_identity
from concourse.tile import add_dep_helper

F32 = mybir.dt.float32
BF16 = mybir.dt.bfloat16
F8 = mybir.dt.float8e4
NPBF = ml_dtypes.bfloat16
NPF8 = ml_dtypes.float8_e4m3
WK_PRESCALE = 1024.0
WQ_PRESCALE = 64.0

NC = 8
BSZ = 16
SEQ = 2048
DIM = 4096
NH = 32
HD = 128
B_LOC = 2        # batches per core
HL = 4           # local heads per core (= one kv group)
N_KV = 8
NT = SEQ // 128  # 16 t-tiles per batch
NDC = DIM // 128 # 32 D-chunks
SCALE = 1.0 / math.sqrt(HD)


def build_program(debug=False, nocc=False, noattn=False, notrans=False, nocopy=False):
    nc = bacc.Bacc("TRN2", target_bir_lowering=False, debug=False)

    xp = nc.dram_tensor("xp", [B_LOC, SEQ, DIM], BF16, kind="ExternalInput")
    # xlT[p, c, b] = x_pre[b, -1, c*128+p]
    xlT = nc.dram_tensor("xlT", [128, NDC * BSZ], BF16, kind="ExternalInput")
    # wqT[p, c*512 + h*128 + o] = wq[512r + h*128 + o, c*128 + p]
    wqT = nc.dram_tensor("wqT", [128, NDC * HL * HD], BF16,
                         kind="ExternalInput")
    # wk_s = wk * SCALE  (full, natural [kv*128+d, D])
    wk = nc.dram_tensor("wk", [N_KV * HD, DIM], BF16, kind="ExternalInput")
    # wvT[p, c*128 + d] = wv[128r + d, c*128 + p]
    wvT = nc.dram_tensor("wvT", [128, NDC * HD], BF16, kind="ExternalInput")
    # woT[p, h*4096 + jc*128 + j] = wo[jc*128 + j, 512r + h*128 + p]
    woT = nc.dram_tensor("woT", [128, HL * DIM], BF16, kind="ExternalInput")
    # yT[p, b*256 + jc*8 + s] = y_partial[2s+b, jc*128+p]
    yT = nc.dram_tensor("yT", [128, B_LOC * NDC * NC], BF16,
                        kind="ExternalOutput")
    if debug:
        dbg_xq = nc.dram_tensor("dbg_xq", [128, 64], BF16, kind="ExternalOutput")
        dbg_qt = nc.dram_tensor("dbg_qt", [64, DIM], BF16, kind="ExternalOutput")
        dbg_qtT = nc.dram_tensor("dbg_qtT", [128, B_LOC * NDC * NH],
                                 BF16, kind="ExternalOutput")
        dbg_ctx = nc.dram_tensor("dbg_ctx", [NH, B_LOC * DIM], BF16,
                                 kind="ExternalOutput")
        dbg_out = nc.dram_tensor("dbg_out", [NH, B_LOC * HD], BF16,
                                 kind="ExternalOutput")

    rg = [list(range(NC))]
    vs_engines = None  # round-robin copy engines, set below

    with tile.TileContext(nc) as tc:
        with (
            tc.tile_pool(name="persist", bufs=1) as pers,
            tc.tile_pool(name="dram", bufs=1, space="DRAM") as dram,
            tc.tile_pool(name="xpool", bufs=6) as xpool,
            tc.tile_pool(name="xTpool", bufs=3) as xTpool,
            tc.tile_pool(name="attn", bufs=8) as apool,
            tc.tile_pool(name="small", bufs=2) as smallp,
            tc.tile_pool(name="ctxsb", bufs=1) as ctxsbp,
            tc.tile_pool(name="pC", bufs=1) as pC,
            tc.tile_pool(name="pCw", bufs=1) as pCw,
            tc.tile_pool(name="tps", bufs=3, space="PSUM") as tps,
            tc.tile_pool(name="scps", bufs=1, space="PSUM") as scps,
                                    tc.tile_pool(name="ctxps", bufs=1, space="PSUM") as ctxps,
            tc.tile_pool(name="sumps", bufs=1, space="PSUM") as sumps,
            tc.tile_pool(name="miscps", bufs=1, space="PSUM") as miscps,
        ):
            fps = miscps
            pCps = miscps
            yps = miscps
            ident = pers.tile([128, 128], BF16)
            make_identity(nc, ident)
            ones_bf = pers.tile([128, 1], BF16)
            nc.vector.memset(ones_bf, 1.0)

            a2a1_in = dram.tile([BSZ, 512], BF16)
            a2a1_out = dram.tile([BSZ, 512], BF16)
            a2a2_in = [dram.tile([NC * HL, DIM], BF16, name=f"a2a2i{b}")
                       for b in range(B_LOC)]
            a2a2_out = [dram.tile([NC * HL, DIM], BF16, name=f"a2a2o{b}")
                        for b in range(B_LOC)]

            # ---------------- Phase A: xq (head-sharded) -> tiny AllToAll
            qtT_all = pers.tile([128, B_LOC * NDC * NH], BF16, name="qtTall")
            qtT = [qtT_all[:, b * NDC * NH:(b + 1) * NDC * NH]
                   for b in range(B_LOC)]
            stage1 = []
            with (
                tc.tile_pool(name="pA", bufs=1) as pA,
                tc.tile_pool(name="pAw", bufs=16) as pAw,
            ):
                xlT_sb = pA.tile([128, NDC * BSZ], BF16)
                nc.sync.dma_start(out=xlT_sb, in_=xlT[:, :])
                wq_pieces = []
                for q in range(16):
                    wq_q = pAw.tile([128, 2 * HL * HD], BF16, tag="wqq",
                                    name=f"wqq{q}")
                    nc.sync.dma_start(
                        out=wq_q, in_=wqT[:, q * 1024:(q + 1) * 1024])
                    wq_pieces.append(wq_q)
                # xq[b, o] in two column-half chains (separate PSUM
                # banks) so half 0's drain copy + staging DMA overlap
                # half 1's matmuls.
                xq_psf0 = scps.tile([128, 512], F32, tag="sc")
                xq_psf1 = miscps.tile([128, 512], F32, tag="ctp")
                for half, psf in ((0, xq_psf0), (1, xq_psf1)):
                    xq_ps = psf[0:BSZ, 0:256]
                    off = half * 256
                    for q in range(16):
                        for k in range(2):
                            c = q * 2 + k
                            nc.tensor.matmul(
                                xq_ps,
                                xlT_sb[:, c * BSZ:(c + 1) * BSZ],
                                wq_pieces[q][:, k * 512 + off:
                                             k * 512 + off + 256],
                                start=(c == 0), stop=(c == NDC - 1))
                    xq_h = pA.tile([BSZ, 256], BF16, tag=f"xqh{half}",
                                   name=f"xqh{half}")
                    nc.scalar.copy(out=xq_h, in_=xq_ps)
                    d = nc.scalar.dma_start(
                        out=a2a1_in[:, off:off + 256], in_=xq_h)
                    stage1.append(d)

            cc1 = None
            if not nocc:
                cc1 = nc.gpsimd.collective_compute(
                    "AllToAll", mybir.AluOpType.bypass,
                    ins=[a2a1_in.opt()], outs=[a2a1_out.opt()],
                    replica_groups=rg)
                for d in stage1:
                    add_dep_helper(cc1.ins, d.ins, reason="a2a1 input ready")

            # qtT[b][p=D, c*32+h] = sum_d xq[2r+b, h*128+d] * wk_s[h*128+d, c*128+p]
            with (
                tc.tile_pool(name="qn", bufs=1) as qn,
                tc.tile_pool(name="wkp", bufs=8) as wkp,
            ):
                # wk pieces per kv group, streamed (SP queue, after wqT)
                wk_dmas = []
                wk_pieces = []
                for kv in range(N_KV):
                    wkq = wkp.tile([HD, DIM], BF16, tag="wkp",
                                   name=f"wk{kv}")
                    dk = nc.sync.dma_start(
                        out=wkq, in_=wk[kv * HD:(kv + 1) * HD, :])

                    wk_dmas.append(dk)
                    wk_pieces.append(wkq)
                xq_loc = qn.tile([B_LOC, DIM], BF16)
                xql_dmas = []
                av = a2a1_out.rearrange("(sq bl) o -> bl sq o", sq=NC)
                for bl in range(B_LOC):
                    d = nc.scalar.dma_start(
                        out=xq_loc[bl:bl + 1].rearrange(
                            "p (sq o) -> p sq o", sq=NC),
                        in_=av[bl])
                    xql_dmas.append(d)
                    if cc1 is not None:
                        add_dep_helper(d.ins, cc1.ins, reason="a2a1 done")
                # xqT2[p=d, 2*hg+bl] via PE transposes
                xqT2_psf = miscps.tile([128, 512], BF16, tag="ctp")
                for c in range(NDC):
                    nc.tensor.transpose(
                        xqT2_psf[:, c * 2:(c + 1) * 2],
                        xq_loc[:, c * 128:(c + 1) * 128],
                        ident[0:B_LOC, 0:B_LOC])
                xqT2_sb = qn.tile([128, NDC * B_LOC], BF16)
                nc.scalar.copy(out=xqT2_sb, in_=xqT2_psf[:, 0:NDC * B_LOC])
                # per kv: qtT chunks [128 D, (c, h, bl)]
                for kv in range(N_KV):
                    if kv % 2 == 0:
                        qt_ps = scps.tile([128, 512], F32, tag="sc")
                    else:
                        qt_ps = miscps.tile([128, 512], F32, tag="ctp")
                    qp = qt_ps.rearrange("p (c h bl) -> p c h bl", c=NDC, h=HL)
                    for c in range(NDC):
                        nc.tensor.matmul(
                            qt_ps[:, c * 8:(c + 1) * 8],
                            wk_pieces[kv][:, c * 128:(c + 1) * 128],
                            xqT2_sb[:, 8 * kv:8 * (kv + 1)],
                            start=True, stop=True)
                    qall = qtT_all.rearrange("p (bl c hh) -> p bl c hh",
                                             bl=B_LOC, c=NDC)
                    if kv % 2 == 0:
                        nc.vector.tensor_copy(
                            out=qall[:, :, :, 4 * kv:4 * (kv + 1)],
                            in_=qt_ps[:, 0:256].rearrange(
                                "p (c h bl) -> p bl c h", c=NDC, h=HL))
                    else:
                        nc.scalar.copy(
                            out=qall[:, :, :, 4 * kv:4 * (kv + 1)],
                            in_=qt_ps[:, 0:256].rearrange(
                                "p (c h bl) -> p bl c h", c=NDC, h=HL))
                if debug:
                    for b in range(B_LOC):
                        nc.sync.dma_start(
                            out=dbg_qtT[:, b * NDC * NH:(b + 1) * NDC * NH],
                            in_=qtT[b])

            # ---------------- Phase B: streaming attention per local batch
            cc2 = [None, None]
            if True:
                xdmas = []
                import os
                _CP = os.environ.get("XTCOPY", "3d1a")
                def xt_copy(g, out, in_):
                    # GPSIMD cannot read PSUM; split PSUM->SBUF copies
                    # between DVE and ACT.
                    if _CP == "alldve":
                        nc.vector.tensor_copy(out=out, in_=in_)
                    elif _CP == "2d2a":
                        if g in (1, 3):
                            nc.scalar.copy(out=out, in_=in_)
                        else:
                            nc.vector.tensor_copy(out=out, in_=in_)
                    elif _CP == "split":
                        if g in (1, 3):
                            nc.scalar.copy(out=out[:, 0:512], in_=in_[:, 0:512])
                            nc.vector.tensor_copy(out=out[:, 512:1024],
                                                  in_=in_[:, 512:1024])
                        else:
                            nc.vector.tensor_copy(out=out, in_=in_)
                    else:
                        if g == 1:
                            nc.scalar.copy(out=out, in_=in_)
                        else:
                            nc.vector.tensor_copy(out=out, in_=in_)
                fin_state = {}
                fin2_state = {}
                fin_stage = {}

                def finalize_part1(bb, ctx_ps_b, sumT_ps_b):
                    sumT_sb = smallp.tile([NH, 1], F32, tag="ssum")
                    nc.vector.tensor_copy(out=sumT_sb, in_=sumT_ps_b)
                    recip = smallp.tile([NH, 1], F32, tag="recip")
                    nc.vector.reciprocal(out=recip, in_=sumT_sb)
                    ctxT_sb = ctxsbp.tile([128, NDC * NH], BF16, tag="ctxT")
                    for half in range(2):
                        nc.scalar.copy(
                            out=ctxT_sb[:, half * 512:(half + 1) * 512],
                            in_=ctx_ps_b[:, half * 512:(half + 1) * 512])
                    fin_state[bb] = (ctxT_sb, recip)

                def finalize_part2_groups(bb, groups, state):
                    last = (bb == B_LOC - 1)
                    ctxT_sb, recip = fin_state[bb]
                    if "ctx_sb0" not in state:
                        state["ctx_sb0"] = ctxsbp.tile([NH, DIM // 2], BF16,
                                                       tag="ctxn0",
                                                       name=f"ctxn0_{bb}")
                        state["ctx_sb1"] = ctxsbp.tile([NH, DIM // 2], BF16,
                                                       tag="ctxn1",
                                                       name=f"ctxn1_{bb}")
                    for g in groups:
                        ctx_sb = (state["ctx_sb0"] if g < 2
                                  else state["ctx_sb1"])
                        goff = (g % 2) * 1024
                        tp2f = tps.tile([128, 1024], BF16, tag="xtp")
                        tp2 = tp2f[0:NH]
                        for k in range(8):
                            c = g * 8 + k
                            nc.tensor.transpose(
                                tp2[:, k * 128:(k + 1) * 128],
                                ctxT_sb[:, c * NH:(c + 1) * NH],
                                ident)
                        if last and g == 1:
                            nc.scalar.mul(
                                out=ctx_sb[:, goff:goff + 1024],
                                in_=tp2, mul=recip)
                        else:
                            nc.vector.tensor_scalar_mul(
                                ctx_sb[:, goff:goff + 1024], tp2, recip)

                def finalize_part2_stage(bb, state, half):
                    last = (bb == B_LOC - 1)
                    ctx_h = state["ctx_sb0" if half == 0 else "ctx_sb1"]
                    dst = a2a2_in[bb][:, half * 2048:(half + 1) * 2048]
                    if last:
                        d = nc.scalar.dma_start(out=dst, in_=ctx_h)
                    else:
                        d = nc.gpsimd.dma_start(out=dst, in_=ctx_h)
                    state.setdefault("stages", []).append(d)

                def finalize_part2_finish(bb, state):
                    fin_state.pop(bb)
                    fin_stage[bb] = state["stages"][-1]
                    if not nocc:
                        cc2[bb] = nc.gpsimd.collective_compute(
                            "AllToAll", mybir.AluOpType.bypass,
                            ins=[a2a2_in[bb].opt()], outs=[a2a2_out[bb].opt()],
                            replica_groups=rg)
                        for d in state["stages"]:
                            add_dep_helper(cc2[bb].ins, d.ins,
                                           reason="a2a2 input ready")

                for b in range(B_LOC):
                    ctx_ps = ctxps.tile([128, NDC * NH], F32, tag="ctx")
                    sumT_ps = sumps.tile([NH, 1], F32, tag="sumT")

                    def emit_attn(tt, x_sb, xT_sb):
                        # alternate scores PSUM between scps and the
                        # attention-idle miscps bank: free double-buffering
                        # that unserializes scores(t+1) from exp(t)'s read.
                        if tt % 2 == 0:
                            sc_full = scps.tile([128, 512], F32, tag="sc")
                        else:
                            sc_full = miscps.tile([128, 512], F32,
                                                  tag="ctp")
                        sc_ps = sc_full[:, 0:NH]
                        for c in range(NDC):
                            nc.tensor.matmul(
                                sc_ps,
                                xT_sb[:, c * 128:(c + 1) * 128],
                                qtT[b][:, c * NH:(c + 1) * NH],
                                start=(c == 0), stop=(c == NDC - 1))
                        at_sb = apool.tile([128, NH], BF16, tag="at")
                        nc.scalar.activation(
                            out=at_sb, in_=sc_ps,
                            func=mybir.ActivationFunctionType.Exp)
                        nc.tensor.matmul(sumT_ps, at_sb, ones_bf,
                                         start=(tt == 0), stop=(tt == NT - 1))
                        bank_start = [None, None]
                        for c in range(NDC):
                            mm = nc.tensor.matmul(
                                ctx_ps[:, c * NH:(c + 1) * NH],
                                x_sb[:, c * 128:(c + 1) * 128],
                                at_sb,
                                start=(tt == 0 and c % 16 == 0),
                                stop=(tt == NT - 1),
                                skip_group_check=True)
                            if tt == 0:
                                if c % 16 == 0:
                                    bank_start[c // 16] = mm
                                else:
                                    add_dep_helper(
                                        mm.ins, bank_start[c // 16].ins,
                                        reason="bank wipe first")

                    import os as _os
                    _LAG = int(_os.environ.get("ALAG", "2"))
                    pending = []
                    for tt in range(NT):
                        x_sb = xpool.tile([128, DIM], BF16, tag="x",
                                          name=f"x{b}_{tt}")
                        xd = nc.sync.dma_start(
                            out=x_sb, in_=xp[b, tt * 128:(tt + 1) * 128, :])
                        import os as _os2
                        _XH = _os2.environ.get("XHOLD", "wk")
                        if len(xdmas) == 0 and _XH == "xql" and xql_dmas:
                            add_dep_helper(xd.ins, xql_dmas[-1].ins,
                                           reason="weights+xql first")
                        elif len(xdmas) == 0 and _XH == "wk" and wk_dmas:
                            add_dep_helper(xd.ins, wk_dmas[-1].ins,
                                           reason="wk first")
                        xdmas.append(xd)
                        xT_sb = xTpool.tile([128, DIM], BF16, tag="xT")
                        for g in range(0 if notrans else 4):
                            tp = tps.tile([128, 1024], BF16, tag="xtp")
                            for k in range(8):
                                c = g * 8 + k
                                nc.tensor.transpose(
                                    tp[:, k * 128:(k + 1) * 128],
                                    x_sb[:, c * 128:(c + 1) * 128], ident)
                            xt_copy(g, xT_sb[:, g * 1024:(g + 1) * 1024], tp)
                        if noattn:
                            continue
                        pending.append((tt, x_sb, xT_sb))
                        if len(pending) > _LAG:
                            emit_attn(*pending.pop(0))
                        if b == 1 and 0 in fin_state and tt == int(__import__('os').environ.get('FIN2', '2')):
                            finalize_part2_groups(0, [0, 1], fin2_state)
                            finalize_part2_stage(0, fin2_state, 0)
                            finalize_part2_groups(0, [2, 3], fin2_state)
                            finalize_part2_stage(0, fin2_state, 1)
                            finalize_part2_finish(0, fin2_state)
                    for p in pending:
                        emit_attn(*p)
                    finalize_part1(b, ctx_ps, sumT_ps)
                    if b == B_LOC - 1:
                        st = {}
                        finalize_part2_groups(b, [0, 1], st)
                        finalize_part2_stage(b, st, 0)
                        finalize_part2_groups(b, [2, 3], st)
                        finalize_part2_stage(b, st, 1)
                        finalize_part2_finish(b, st)

                # ---------------- Phase C: output projection per batch slot
                wvT_sb = pCw.tile([128, NDC * HD], BF16)
                dwv = nc.sync.dma_start(out=wvT_sb, in_=wvT[:, :])
                woT_sb = pCw.tile([128, HL * DIM], BF16)
                dwo = nc.sync.dma_start(out=woT_sb, in_=woT[:, :])
                import os as _os3
                _WVM = _os3.environ.get("WVMODE", "mid")
                if _WVM == "tail" and (B_LOC - 1) in fin_stage:
                    add_dep_helper(dwv.ins, fin_stage[B_LOC - 1].ins,
                                   reason="wv in collective window")
                    add_dep_helper(dwo.ins, fin_stage[B_LOC - 1].ins,
                                   reason="wo in collective window")
                else:
                    _WVK = int(_os3.environ.get("WVK", str(NT + 4)))
                    add_dep_helper(dwv.ins, xdmas[_WVK].ins,
                                   reason="late wv")
                    add_dep_helper(dwo.ins, xdmas[_WVK + 4].ins,
                                   reason="late wo")
                yT_sb = pCw.tile([128, NDC * B_LOC * NC], BF16)
                for b in range(B_LOC):
                    ctxgf = xpool.tile([128, DIM], BF16, tag="x",
                                       name=f"ctxg{b}")
                    ctxg = ctxgf[0:NH]
                    d = nc.scalar.dma_start(out=ctxg, in_=a2a2_out[b][:, :])
                    if cc2[b] is not None:
                        add_dep_helper(d.ins, cc2[b].ins,
                                       reason="a2a2 done")
                    ctxgT = pC.tile([128, NDC * NH], BF16, tag="ctxgT")
                    for hf in range(2):
                        tpg = tps.tile([128, 1024], BF16, tag="xtp",
                                       name=f"tpg{b}_{hf}")
                        for k in range(16):
                            c = hf * 16 + k
                            nc.tensor.transpose(
                                tpg[:, k * 32:(k + 1) * 32],
                                ctxg[:, c * 128:(c + 1) * 128],
                                ident[0:NH, 0:NH])
                        nc.vector.tensor_copy(
                            out=ctxgT[:, hf * 512:(hf + 1) * 512],
                            in_=tpg[:, 0:512])
                    # outT[d, (s,h)] = sum_D wvT[D, d]^T ctxgT[D, (s,h)]
                    op_ps = pCps.tile([HD, NH], F32, tag="ctp")
                    for c in range(NDC):
                        nc.tensor.matmul(op_ps,
                                         wvT_sb[:, c * 128:(c + 1) * 128],
                                         ctxgT[:, c * NH:(c + 1) * NH],
                                         start=(c == 0), stop=(c == NDC - 1))
                    outT = pC.tile([128, NH], BF16, tag="outT")
                    nc.vector.tensor_copy(out=outT[0:HD], in_=op_ps)
                    # yT[j, s] = sum_h sum_d woT[d, (h, jc, j)] * outT[d, (s, h)]
                    ov = outT.rearrange("p (s h) -> p h s", h=HL)
                    y_ps = yps.tile([128, NDC * NC], F32, tag="ctp")
                    for jc in range(NDC):
                        for h in range(HL):
                            nc.tensor.matmul(
                                y_ps[:, jc * NC:(jc + 1) * NC],
                                woT_sb[:, h * DIM + jc * 128:
                                       h * DIM + (jc + 1) * 128],
                                ov[:, h, :],
                                start=(h == 0), stop=(h == HL - 1))
                    yv = yT_sb.rearrange("p (b jcs) -> b p jcs", b=B_LOC)
                    nc.vector.tensor_copy(out=yv[b], in_=y_ps)
                    nc.sync.dma_start(
                        out=yT.rearrange("p (b jcs) -> b p jcs", b=B_LOC)[b],
                        in_=yv[b])

    nc.finalize()
    return nc


_PROGRAM_CACHE = {}


def _prep_inputs(x_pre, wq, wk, wv, wo):
    """Shard + cast + pre-transpose on host. Returns in_maps for 8 cores."""
    xlT_full = np.ascontiguousarray(
        x_pre[:, -1, :].T.astype(NPBF))                    # [4096, 16]
    xlT_full = xlT_full.reshape(NDC, 128, BSZ).transpose(1, 0, 2)  # [128,c,b]
    xlT_flat = np.ascontiguousarray(xlT_full.reshape(128, NDC * BSZ))

    wk_s = (wk * SCALE).astype(NPBF)
    in_maps = []
    for r in range(NC):
        # wqT[p, c, h, o] = wq[512r + h*128 + o, c*128 + p]
        wq_sl = wq[512 * r:512 * (r + 1), :].astype(NPBF)   # [512, 4096]
        wqT_r = wq_sl.reshape(HL, 128, NDC, 128).transpose(3, 2, 0, 1)
        wqT_r = np.ascontiguousarray(wqT_r.reshape(128, NDC * HL * HD))
        # wvT[p, c, d] = wv[128r + d, c*128 + p]
        wv_sl = wv[128 * r:128 * (r + 1), :].astype(NPBF)   # [128 d, 4096 D]
        wvT_r = wv_sl.reshape(128, NDC, 128).transpose(2, 1, 0)
        wvT_r = np.ascontiguousarray(wvT_r.reshape(128, NDC * HD))
        # woT[p, h, jc, j] = wo[jc*128 + j, 512r + h*128 + p]
        wo_sl = wo[:, 512 * r:512 * (r + 1)].astype(NPBF)   # [4096 j, 512 o]
        woT_r = wo_sl.reshape(NDC, 128, HL, 128).transpose(3, 2, 0, 1)
        woT_r = np.ascontiguousarray(woT_r.reshape(128, HL * DIM))
        in_maps.append({
            "xp": np.ascontiguousarray(x_pre[2 * r:2 * r + 2].astype(NPBF)),
            "xlT": xlT_flat,
            "wqT": wqT_r,
            "wk": np.ascontiguousarray(wk_s),
            "wvT": wvT_r,
            "woT": woT_r,
        })
    return in_maps


def kernel(x_pre, wq, wk, wv, wo, _trace=False, _tmpdir=None, _debug=False):
    x_pre = np.asarray(x_pre, dtype=np.float32)
    wq = np.asarray(wq, dtype=np.float32)
    wk = np.asarray(wk, dtype=np.float32)
    wv = np.asarray(wv, dtype=np.float32)
    wo = np.asarray(wo, dtype=np.float32)

    key = "nc_dbg" if _debug else "nc"
    if key not in _PROGRAM_CACHE:
        _PROGRAM_CACHE[key] = build_program(debug=_debug)
        _PROGRAM_CACHE["nc"] = _PROGRAM_CACHE[key]
    nc = _PROGRAM_CACHE[key]

    in_maps = _prep_inputs(x_pre, wq, wk, wv, wo)

    kwargs = {}
    if _trace:
        kwargs = dict(trace=True, trace_cores=[0])
    if _tmpdir is not None:
        kwargs["tmpdir"] = _tmpdir
    res = run_bass_kernel_spmd(nc, in_maps, core_ids=list(range(NC)), **kwargs)

    y = np.zeros((BSZ, DIM), np.float64)
    for r in range(NC):
        yT_r = np.asarray(res.results[r]["yT"], np.float32)
        yT_r = yT_r.reshape(128, B_LOC, NDC, NC)
        # y[2s+b, jc*128+p] += yT_r[p, b, jc, s]
        y += yT_r.transpose(3, 1, 2, 0).reshape(BSZ, DIM)
    if _debug:
        _PROGRAM_CACHE["dbg"] = res
    if _trace:
        print("HW exec time:", res.exec_time_ns, "ns")
    return y.astype(np.float32).reshape(BSZ, 1, DIM)

